# revision 22
# baseline (speedup 1.0000x reference)
"""Trainium2 Bass kernel for nn_Block_Head_83047487635490 (Mamba motion block).

Sharding: 8 cores = 2 batch groups x 4-way tensor-parallel (d_inner 2048 -> 512
per core).  Per group: mamba in_proj/conv/x_proj/dt_proj/scan/out_proj sharded
over d_inner with a small fp16 all-reduce of x_db (288 x L) and a
reduce-scatter of out_proj partials over tokens; the gated MLPs and the text
MLP run token-parallel (full hidden) on the scattered quarters.

Selective scan: per (chunk, dtile, n-group) the recurrence
h = exp(delta*A_n) * h + (delta*u) * B_n runs as a single fp16
tensor_tensor_scan over Gn concatenated n-blocks (decay zeroed at block
starts resets the state); y = sum_n h*C_n via PE identity-matmul accumulation
in PSUM.
"""

import sys

if "/opt/trn_rl_repo" not in sys.path:
    sys.path.insert(0, "/opt/trn_rl_repo")

import numpy as np

from concourse import bacc, bass, mybir, tile
from concourse.bass_utils import run_bass_kernel_spmd

# ---------------------------------------------------------------- constants
B = 2
T = 512
N_TEXT = 128
TOTAL = 2048          # 3*T + 4*N_TEXT
D_MODEL = 512
D_STATE = 128
D_CONV = 4
D_INNER = 2048
DT_RANK = 32
HID_SMALL = 768
HID_TEXT = 2048
EPS = 1e-6
OUT_TOTAL = 2560      # music 512 + up 512 + down 1024 + text 512

R_SH = 4              # TP degree
DI_SH = D_INNER // R_SH
NDT = DI_SH // 128    # 4 d-tiles per core
F = 512               # time chunk
GN = 4                # n values per scan instruction
NG = D_STATE // GN    # 32
BLK = GN * F

BRANCHES = [(0, 512, 1), (512, 512, 1), (1024, 1024, 2)]  # (tok_off, L, chunks)
MLP_KEYS = ["gate_mlp_1", "gate_mlp_2", "gate_mlp_3"]
TEXT_SEGS = ["text_upper", "text_lower", "text_torso", "text_whole"]
MAMBA_KEYS = ["mamba_music", "mamba_up", "mamba_down"]

FP16 = mybir.dt.float16
FP32 = mybir.dt.float32
AF = mybir.ActivationFunctionType
ALU = mybir.AluOpType

OUT_SECS = [(0, 128), (128, 128), (256, 256), (512, 128)]
GROUPS = [[0, 1, 2, 3], [4, 5, 6, 7]]


def build_program():
    nc = bacc.Bacc("TRN2", target_bir_lowering=False, debug=False, num_devices=8)

    def din(name, shape, dt=FP16):
        return nc.dram_tensor(name, list(shape), dt, kind="ExternalInput").ap()

    x_t = din("x_t", (16, 128, 512))
    xq = din("xq", (640, 512), FP32)
    ident_in = din("ident", (128, 128))
    A_in = din("A_bc", (NDT, 128, 128), FP32)
    dtb_in = din("dtb", (3, NDT, 128, 1), FP32)
    D_in = din("D_sh", (3, NDT, 128, 1), FP32)
    w_in = [din(f"w_in_{m}", (4, 128, 1024)) for m in range(3)]
    w_conv = [din(f"w_conv_{m}", (NDT, 128, D_CONV), FP32) for m in range(3)]
    w_xp = [din(f"w_xp_{m}", (NDT, 128, 288)) for m in range(3)]
    w_dt = [din(f"w_dt_{m}", (DT_RANK, DI_SH)) for m in range(3)]
    w_out = [din(f"w_out_{m}", (NDT, 128, 512)) for m in range(3)]
    w_fc1 = [din(f"w_fc1_{m}", (4, 128, 2 * HID_SMALL)) for m in range(3)]
    w_fc2 = [din(f"w_fc2_{m}", (HID_SMALL // 128, 128, 512)) for m in range(3)]
    w_fc1_t = din("w_fc1_t", (4, 128, 2 * HID_TEXT))
    w_fc2_t = din("w_fc2_t", (HID_TEXT // 128, 128, 512))
    out_d = nc.dram_tensor("out", [640, 512], FP32, kind="ExternalOutput").ap()

    with tile.TileContext(nc) as tc:
        with (
            tc.tile_pool(name="const", bufs=1) as constp,
            tc.tile_pool(name="persist", bufs=1) as persist,
            tc.tile_pool(name="small", bufs=2) as small,
            tc.tile_pool(name="cols", bufs=4) as colsp,
            tc.tile_pool(name="wpool", bufs=4) as wpool,
            tc.tile_pool(name="wmlp", bufs=6) as wmlpp,
            tc.tile_pool(name="branch", bufs=1) as branchp,
            tc.tile_pool(name="scan", bufs=3) as scanp,
            tc.tile_pool(name="bc", bufs=3) as bcp,
            tc.tile_pool(name="post", bufs=2) as postp,
            tc.tile_pool(name="mlp", bufs=1) as mlpp,
            tc.tile_pool(name="hpool", bufs=16) as hpoolp,
            tc.tile_pool(name="psmm", bufs=4, space="PSUM") as psmm,
            tc.tile_pool(name="ypsum", bufs=1, space="PSUM") as ypsum,
            tc.tile_pool(name="dram", bufs=1, space="DRAM") as dramp,
        ):
            # ---------------- constants
            identT = constp.tile([128, 128], FP16, tag="ident")
            nc.sync.dma_start(out=identT[:], in_=ident_in[:])
            epscol = constp.tile([128, 1], FP32, tag="epscol")
            nc.vector.memset(epscol[:], EPS)
            A_t, dtb_t, D_t = [], {}, {}
            for dt in range(NDT):
                a = constp.tile([128, 128], FP32, tag=f"A{dt}")
                nc.sync.dma_start(out=a[:], in_=A_in[dt])
                A_t.append(a)
            for m in range(3):
                for dt in range(NDT):
                    bcol = constp.tile([128, 1], FP32, tag=f"dtb{m}_{dt}")
                    nc.sync.dma_start(out=bcol[:], in_=dtb_in[m, dt])
                    dtb_t[(m, dt)] = bcol
                    dcol = constp.tile([128, 1], FP32, tag=f"D{m}_{dt}")
                    nc.sync.dma_start(out=dcol[:], in_=D_in[m, dt])
                    D_t[(m, dt)] = dcol

            # ---------------- helpers
            def rms_tile(src_tile, dst_tile):
                scratch = small.tile([128, 512], FP16, tag="rms_scratch", bufs=1)
                ssq = colsp.tile([128, 1], FP32, tag="rms_ssq")
                nc.scalar.activation(out=scratch[:], in_=src_tile[:],
                                     func=AF.Square, accum_out=ssq[:])
                rr = colsp.tile([128, 1], FP32, tag="rms_rr")
                nc.scalar.activation(out=rr[:], in_=ssq[:], func=AF.Sqrt,
                                     scale=1.0 / D_MODEL, bias=epscol[:])
                inv = colsp.tile([128, 1], FP32, tag="rms_inv")
                nc.vector.reciprocal(out=inv[:], in_=rr[:])
                nc.vector.tensor_scalar(out=dst_tile[:], in0=src_tile[:],
                                        scalar1=inv[:], scalar2=None,
                                        op0=ALU.mult)

            def transpose_into(src_tile, n_blocks, put_block):
                for bb in range(n_blocks):
                    pst = psmm.tile([128, 128], FP16, tag="ps")
                    nc.tensor.transpose(pst[:], src_tile[:, bb * 128:(bb + 1) * 128],
                                        identT[:])
                    put_block(bb, pst)

            # ---------------- prolog: x -> rmsnorm -> transpose -> xnT
            xnT = [persist.tile([128, TOTAL], FP16, tag=f"xnT{k}",
                                name=f"xnT{k}") for k in range(4)]
            for i in range(16):
                xt = postp.tile([128, 512], FP16, tag="so", bufs=4, name="xt")
                nc.sync.dma_start(out=xt[:], in_=x_t[i])
                xn = postp.tile([128, 512], FP16, tag="y2", name="xn")
                rms_tile(xt, xn)

                def put(kt, pst, i=i):
                    nc.scalar.copy(out=xnT[kt][:, i * 128:(i + 1) * 128],
                                   in_=pst[:])
                transpose_into(xn, 4, put)

            # ---------------- generic gated-MLP block on one (128, 512) resid
            def mlp_block(resid_tile, w1_ap, hid, w2_ap, out_row):
                xn2 = mlpp.tile([128, 512], FP16, tag="mlp_xn")
                rms_tile(resid_tile, xn2)
                xn2T = []
                for kt in range(4):
                    dstt = mlpp.tile([128, 128], FP16, tag=f"mlp_xnT{kt}")
                    xn2T.append(dstt)

                def putx(kt, pst):
                    nc.scalar.copy(out=xn2T[kt][:], in_=pst[:])
                transpose_into(xn2, 4, putx)

                n_h = hid // 128
                hh = []
                for oy in range(n_h):
                    ps_y = psmm.tile([128, 128], FP32, tag="ps")
                    ps_g = psmm.tile([128, 128], FP32, tag="ps")
                    for half, ps in ((0, ps_y), (1, ps_g)):
                        o = oy + half * n_h
                        for kt in range(4):
                            w = wmlpp.tile([128, 128], FP16, tag="wm1")
                            nc.sync.dma_start(
                                out=w[:], in_=w1_ap[kt][:, o * 128:(o + 1) * 128])
                            nc.tensor.matmul(ps[:], w[:], xn2T[kt][:],
                                             start=(kt == 0), stop=(kt == 3))
                    sg = mlpp.tile([128, 128], FP16, tag="mlp_sg")
                    nc.scalar.activation(out=sg[:], in_=ps_g[:], func=AF.Sigmoid)
                    sg2 = mlpp.tile([128, 128], FP16, tag="mlp_sg2")
                    nc.vector.tensor_tensor(out=sg2[:], in0=ps_g[:], in1=sg[:],
                                            op=ALU.mult)
                    ht = hpoolp.tile([128, 128], FP16, tag="mlp_h")
                    nc.vector.tensor_tensor(out=ht[:], in0=ps_y[:], in1=sg2[:],
                                            op=ALU.mult)
                    hh.append(ht)
                mT = mlpp.tile([128, 512], FP16, tag="mlp_mT")
                for ot in range(4):
                    ps = psmm.tile([128, 128], FP32, tag="ps")
                    for kt in range(n_h):
                        w = wmlpp.tile([128, 128], FP16, tag="wm2")
                        nc.sync.dma_start(
                            out=w[:], in_=w2_ap[kt][:, ot * 128:(ot + 1) * 128])
                        nc.tensor.matmul(ps[:], w[:], hh[kt][:],
                                         start=(kt == 0), stop=(kt == n_h - 1))
                    mo = mlpp.tile([128, 128], FP16, tag="mlp_mo")
                    nc.scalar.copy(out=mo[:], in_=ps[:])
                    pst = psmm.tile([128, 128], FP16, tag="ps")
                    nc.tensor.transpose(pst[:], mo[:], identT[:])
                    nc.scalar.copy(out=mT[:, ot * 128:(ot + 1) * 128], in_=pst[:])
                fin = mlpp.tile([128, 512], FP32, tag="mlp_fin")
                nc.vector.tensor_tensor(out=fin[:], in0=mT[:], in1=resid_tile[:],
                                        op=ALU.add)
                nc.sync.dma_start(out=out_d[out_row:out_row + 128, :], in_=fin[:])

            # ---------------- per-branch state
            ar_out = {}
            u_tiles = {}
            du_tiles = {}
            delta_all = {}
            carry = {}
            rs_in = {}
            rs_out = {}
            for m in range(3):
                L = BRANCHES[m][1]
                rs_in[m] = dramp.tile([L, 512], FP16, tag=f"rsin{m}",
                                      name=f"rsin{m}")
                for ch in range(L // 512):
                    rs_out[(m, ch)] = dramp.tile(
                        [128, 512], FP16, tag=f"rsout{m}_{ch}",
                        name=f"rsout{m}_{ch}")

            # ================= pre-scan stage
            def pre_branch(m):
                tok_off, L, n_ch = BRANCHES[m]
                wi = []
                for kt in range(4):
                    wt = wpool.tile([128, 512], FP16, tag="w_in")
                    nc.sync.dma_start(out=wt[:], in_=w_in[m][kt][:, 0:512])
                    wi.append(wt)
                wxp = []
                for kt in range(NDT):
                    wt = wpool.tile([128, 288], FP16, tag="w_xp")
                    nc.sync.dma_start(out=wt[:], in_=w_xp[m][kt])
                    wxp.append(wt)
                wdt = wpool.tile([DT_RANK, DI_SH], FP16, tag="w_dt")
                nc.sync.dma_start(out=wdt[:], in_=w_dt[m][:])
                wcv = []
                for dt in range(NDT):
                    wt = wpool.tile([128, D_CONV], FP32, tag="w_conv")
                    nc.sync.dma_start(out=wt[:], in_=w_conv[m][dt])
                    wcv.append(wt)

                xiT = [branchp.tile([128, L + 3], FP16, tag=f"xiT{dt}_{m}",
                                    name=f"xiT{dt}_{m}") for dt in range(NDT)]
                uu = [xiT[dt][:, 3:3 + L] for dt in range(NDT)]
                duu = [branchp.tile([128, L], FP16, tag=f"du{dt}_{m}",
                                    name=f"du{dt}_{m}") for dt in range(NDT)]
                dall = branchp.tile([128, NDT * L], FP16, tag=f"delta_{m}",
                                    name=f"delta_{m}")

                for dt in range(NDT):
                    nc.vector.memset(xiT[dt][:, 0:3], 0.0)

                # in_proj (xi half only; z recomputed at post time)
                for ch in range(n_ch):
                    c0 = tok_off + ch * F
                    for ot in range(4):
                        ps = psmm.tile([128, 512], FP32, tag="ps")
                        for kt in range(4):
                            nc.tensor.matmul(ps[:], wi[kt][:, ot * 128:(ot + 1) * 128],
                                             xnT[kt][:, c0:c0 + F],
                                             start=(kt == 0), stop=(kt == 3))
                        nc.scalar.copy(
                            out=xiT[ot][:, 3 + ch * F:3 + (ch + 1) * F], in_=ps[:])

                # conv + silu -> u
                for dt in range(NDT):
                    acc0 = small.tile([128, 1024], FP16, tag="conv_a", bufs=1)
                    acc1 = small.tile([128, 1024], FP16, tag="conv_b", bufs=1)
                    nc.vector.tensor_scalar(out=acc0[:, 0:L], in0=xiT[dt][:, 0:L],
                                            scalar1=wcv[dt][:, 0:1], scalar2=None,
                                            op0=ALU.mult)
                    a, b_ = acc0, acc1
                    for j in range(1, D_CONV):
                        nc.vector.scalar_tensor_tensor(
                            out=b_[:, 0:L], in0=xiT[dt][:, j:j + L],
                            scalar=wcv[dt][:, j:j + 1], in1=a[:, 0:L],
                            op0=ALU.mult, op1=ALU.add)
                        a, b_ = b_, a
                    sgu = small.tile([128, 1024], FP16, tag="sgu", bufs=1)
                    nc.scalar.activation(out=sgu[:, 0:L], in_=a[:, 0:L],
                                         func=AF.Sigmoid)
                    nc.vector.tensor_tensor(out=uu[dt][:, 0:L], in0=a[:, 0:L],
                                            in1=sgu[:, 0:L], op=ALU.mult)

                # x_proj partials -> DRAM -> AllReduce (fp16)
                arin = dramp.tile([288, L], FP16, tag=f"arin{m}")
                arout = dramp.tile([288, L], FP16, tag=f"arout{m}")
                for ch in range(n_ch):
                    for po, pw in ((0, 128), (128, 128), (256, 32)):
                        ps = psmm.tile([128, 512], FP32, tag="ps")
                        for kt in range(NDT):
                            nc.tensor.matmul(
                                ps[:pw, :], wxp[kt][:, po:po + pw],
                                uu[kt][:, ch * F:(ch + 1) * F],
                                start=(kt == 0), stop=(kt == NDT - 1))
                        sb = small.tile([128, 512], FP16, tag="xdb_sb")
                        nc.scalar.copy(out=sb[:pw, :], in_=ps[:pw, :])
                        nc.sync.dma_start(
                            out=arin[po:po + pw, ch * F:(ch + 1) * F],
                            in_=sb[:pw, :])
                nc.gpsimd.collective_compute(
                    "AllReduce", ALU.add, replica_groups=GROUPS,
                    ins=[arin.opt()], outs=[arout.opt()])

                # dt_proj + softplus -> delta ; du = delta * u
                dtT = small.tile([DT_RANK, 1024], FP16, tag="dtT", bufs=1)
                nc.sync.dma_start(out=dtT[:, 0:L], in_=arout[0:DT_RANK, :])
                for dt in range(NDT):
                    for ch in range(n_ch):
                        ps = psmm.tile([128, 512], FP32, tag="ps")
                        nc.tensor.matmul(ps[:], wdt[:, dt * 128:(dt + 1) * 128],
                                         dtT[:, ch * F:(ch + 1) * F],
                                         start=True, stop=True)
                        spe = small.tile([128, 512], FP16, tag="spe", bufs=1)
                        nc.scalar.activation(out=spe[:], in_=ps[:], func=AF.Exp,
                                             bias=dtb_t[(m, dt)][:])
                        nc.scalar.activation(
                            out=dall[:, dt * L + ch * F: dt * L + (ch + 1) * F],
                            in_=spe[:], func=AF.Ln, bias=1.0)
                    nc.vector.tensor_tensor(out=duu[dt][:, 0:L],
                                            in0=dall[:, dt * L:dt * L + L],
                                            in1=uu[dt][:, 0:L], op=ALU.mult)

                ar_out[m] = arout
                u_tiles[m] = uu
                du_tiles[m] = duu
                delta_all[m] = dall
                if m == 2:
                    carry[m] = [branchp.tile([128, 128], FP16, tag=f"carry{dt}",
                                             name=f"carry{dt}")
                                for dt in range(NDT)]

            # ================= scan + post of one chunk
            def scan_chunk(m, ch):
                tok_off, L, n_ch = BRANCHES[m]
                arout = ar_out[m]
                dall = delta_all[m]
                duu = du_tiles[m]
                chained = (m == 2 and ch == 1)
                save_carry = (m == 2 and ch == 0)

                y_ps = [ypsum.tile([128, 512], FP32, tag=f"yps{dt}",
                                   name=f"yps{dt}_{m}_{ch}")
                        for dt in range(NDT)]
                for ng in range(NG):
                    n0 = ng * GN
                    b_bc = bcp.tile([128, BLK], FP16, tag="b_bc")
                    src = arout[DT_RANK + n0:DT_RANK + n0 + GN,
                                ch * F:(ch + 1) * F]
                    nc.sync.dma_start(
                        out=b_bc[:].rearrange("p (g f) -> p g f", g=GN),
                        in_=src.unsqueeze(0).broadcast_to([128, GN, F]))
                    c_bc = bcp.tile([128, BLK], FP16, tag="c_bc")
                    src = arout[DT_RANK + D_STATE + n0:
                                DT_RANK + D_STATE + n0 + GN,
                                ch * F:(ch + 1) * F]
                    nc.sync.dma_start(
                        out=c_bc[:].rearrange("p (g f) -> p g f", g=GN),
                        in_=src.unsqueeze(0).broadcast_to([128, GN, F]))

                    for dt in range(NDT):
                        dA = scanp.tile([128, BLK], FP16, tag="dA", bufs=2)
                        dA_g = dA[:].rearrange("p (g f) -> p g f", g=GN)
                        for j in range(GN):
                            nc.scalar.activation(
                                out=dA_g[:, j, :],
                                in_=dall[:, dt * L + ch * F:
                                         dt * L + (ch + 1) * F],
                                func=AF.Exp,
                                scale=A_t[dt][:, n0 + j:n0 + j + 1])
                        bb = scanp.tile([128, BLK], FP16, tag="bb")
                        du_view = duu[dt][:, ch * F:(ch + 1) * F]
                        nc.vector.tensor_tensor(
                            out=bb[:].rearrange("p (g f) -> p g f", g=GN),
                            in0=du_view.unsqueeze(1).broadcast_to([128, GN, F]),
                            in1=b_bc[:].rearrange("p (g f) -> p g f", g=GN),
                            op=ALU.mult)
                        if chained:
                            tmp = colsp.tile([128, GN], FP16, tag="fix")
                            nc.vector.tensor_tensor(
                                out=tmp[:], in0=dA_g[:, :, 0],
                                in1=carry[2][dt][:, n0:n0 + GN], op=ALU.mult)
                            bst = bb[:].rearrange("p (g f) -> p g f", g=GN)[:, :, 0]
                            nc.vector.tensor_tensor(out=bst, in0=tmp[:],
                                                    in1=bst, op=ALU.add)
                        nc.vector.memset(dA_g[:, :, 0], 0.0)
                        hh = scanp.tile([128, BLK], FP16, tag="hh")
                        nc.vector.tensor_tensor_scan(
                            out=hh[:], data0=dA[:], data1=bb[:], initial=0.0,
                            op0=ALU.mult, op1=ALU.add)
                        if save_carry:
                            nc.vector.tensor_copy(
                                out=carry[2][dt][:, n0:n0 + GN],
                                in_=hh[:].rearrange("p (g f) -> p g f",
                                                    g=GN)[:, :, F - 1])
                        prod = scanp.tile([128, BLK], FP16, tag="prod")
                        nc.vector.tensor_tensor(
                            out=prod[:], in0=hh[:], in1=c_bc[:], op=ALU.mult)
                        for g in range(GN):
                            nc.tensor.matmul(
                                y_ps[dt][:], identT[:],
                                prod[:, g * F:(g + 1) * F],
                                start=(ng == 0 and g == 0),
                                stop=(ng == NG - 1 and g == GN - 1))

                # ---- post: z -> silu ; y3 = (y + D*u) * silu(z) ; out_proj
                c0 = tok_off + ch * F
                wiz = []
                for kt in range(4):
                    wt = wpool.tile([128, 512], FP16, tag="w_z")
                    nc.sync.dma_start(out=wt[:], in_=w_in[m][kt][:, 512:1024])
                    wiz.append(wt)
                y3s = []
                for dt in range(NDT):
                    psz = psmm.tile([128, 512], FP32, tag="ps")
                    for kt in range(4):
                        nc.tensor.matmul(psz[:], wiz[kt][:, dt * 128:(dt + 1) * 128],
                                         xnT[kt][:, c0:c0 + F],
                                         start=(kt == 0), stop=(kt == 3))
                    sgz = postp.tile([128, 512], FP16, tag="sgz")
                    nc.scalar.activation(out=sgz[:], in_=psz[:], func=AF.Sigmoid)
                    szt = postp.tile([128, 512], FP16, tag="sz")
                    nc.vector.tensor_tensor(out=szt[:], in0=psz[:], in1=sgz[:],
                                            op=ALU.mult)
                    y2 = postp.tile([128, 512], FP16, tag="y2")
                    nc.vector.scalar_tensor_tensor(
                        out=y2[:], in0=u_tiles[m][dt][:, ch * F:(ch + 1) * F],
                        scalar=D_t[(m, dt)][:], in1=y_ps[dt][:],
                        op0=ALU.mult, op1=ALU.add)
                    y3 = postp.tile([128, 512], FP16, tag="y3", bufs=4)
                    nc.vector.tensor_tensor(out=y3[:], in0=y2[:], in1=szt[:],
                                            op=ALU.mult)
                    y3s.append(y3)

                wo = []
                for kt in range(NDT):
                    wt = wpool.tile([128, 512], FP16, tag="w_out")
                    nc.sync.dma_start(out=wt[:], in_=w_out[m][kt])
                    wo.append(wt)
                so = []
                for ot in range(4):
                    ps = psmm.tile([128, 512], FP32, tag="ps")
                    for kt in range(NDT):
                        nc.tensor.matmul(ps[:], wo[kt][:, ot * 128:(ot + 1) * 128],
                                         y3s[kt][:], start=(kt == 0),
                                         stop=(kt == NDT - 1))
                    st = postp.tile([128, 512], FP16, tag="so", bufs=4)
                    nc.scalar.copy(out=st[:], in_=ps[:])
                    so.append(st)
                for tt in range(4):
                    ob = postp.tile([128, 512], FP16, tag="obuf")
                    for ot in range(4):
                        pst = psmm.tile([128, 128], FP16, tag="ps")
                        nc.tensor.transpose(pst[:],
                                            so[ot][:, tt * 128:(tt + 1) * 128],
                                            identT[:])
                        nc.scalar.copy(out=ob[:, ot * 128:(ot + 1) * 128],
                                       in_=pst[:])
                    nc.sync.dma_start(
                        out=rs_in[m][ch * F + tt * 128:ch * F + (tt + 1) * 128, :],
                        in_=ob[:])

            # ================= post-RS MLP (one 512-token chunk of branch m)
            def mlp_branch(m, ch=0):
                nc.gpsimd.collective_compute(
                    "ReduceScatter", ALU.add, replica_groups=GROUPS,
                    ins=[rs_in[m][ch * 512:(ch + 1) * 512, :]],
                    outs=[rs_out[(m, ch)].opt()])
                sec0, nrow = OUT_SECS[m]
                rsv = mlpp.tile([128, 512], FP16, tag="rsv")
                nc.sync.dma_start(out=rsv[:], in_=rs_out[(m, ch)][:])
                xqv = mlpp.tile([128, 512], FP32, tag="xqv")
                nc.sync.dma_start(
                    out=xqv[:],
                    in_=xq[sec0 + ch * 128:sec0 + (ch + 1) * 128, :])
                resid = mlpp.tile([128, 512], FP32, tag="resid")
                nc.vector.tensor_tensor(out=resid[:], in0=rsv[:], in1=xqv[:],
                                        op=ALU.add)
                mlp_block(resid, w_fc1[m], HID_SMALL, w_fc2[m],
                          sec0 + ch * 128)

            # ---------------- schedule (program order guides the Tile
            # scheduler: each branch's AllReduce flies while the previous
            # branch's scan keeps the Vector engine saturated)
            pre_branch(0)
            xq_text = mlpp.tile([128, 512], FP32, tag="xq_text")
            nc.sync.dma_start(out=xq_text[:], in_=xq[512:640, :])
            mlp_block(xq_text, w_fc1_t, HID_TEXT, w_fc2_t, 512)
            pre_branch(1)
            pre_branch(2)
            scan_chunk(0, 0)
            scan_chunk(1, 0)
            mlp_branch(0)
            scan_chunk(2, 0)
            mlp_branch(1)
            mlp_branch(2, 0)
            scan_chunk(2, 1)
            mlp_branch(2, 1)

    nc.finalize()
    return nc


_PROGRAM = None


def _get_program():
    global _PROGRAM
    if _PROGRAM is None:
        _PROGRAM = build_program()
    return _PROGRAM


def _np(a, dt=np.float32):
    return np.ascontiguousarray(np.asarray(a), dtype=dt)


def prepare_in_maps(x, params):
    x = _np(x)
    norms = {k: _np(v) for k, v in params["norms"].items()}
    mamba = [{k: _np(v) for k, v in params[mk].items()} for mk in MAMBA_KEYS]
    mlps = [{k: _np(v) for k, v in params[mk].items()} for mk in MLP_KEYS]
    textp = {k: _np(v) for k, v in params["text_mlp"].items()}
    ident = np.eye(128, dtype=np.float16)
    norm_fold = [
        (norms["music_1"], norms["music_2"]),
        (norms["up_1"], norms["up_2"]),
        (norms["down_1"], norms["down_2"]),
    ]
    A_neg = -np.exp(mamba[0]["A_log"])
    for mmx in mamba[1:]:
        assert np.allclose(-np.exp(mmx["A_log"]), A_neg), \
            "mamba A_log differ across branches; not supported"
    for mm in mamba:
        assert np.all(mm["conv_b"] == 0), "conv bias not folded"
    for gm in mlps + [textp]:
        assert np.all(gm["fc1_b"] == 0) and np.all(gm["fc2_b"] == 0)

    in_maps = []
    for g in range(2):
        xg = x[g]
        x_t16 = np.ascontiguousarray(
            xg.astype(np.float16).reshape(16, 128, 512))
        for r in range(R_SH):
            d0 = r * DI_SH
            im = {"x_t": x_t16, "ident": ident}
            im["xq"] = np.ascontiguousarray(np.concatenate([
                xg[r * 128:(r + 1) * 128],
                xg[512 + r * 128:512 + (r + 1) * 128],
                xg[1024 + r * 128:1024 + (r + 1) * 128],
                xg[1536 + r * 128:1536 + (r + 1) * 128],
                xg[1536 + r * 128:1536 + (r + 1) * 128],
            ], axis=0), dtype=np.float32)
            im["A_bc"] = np.ascontiguousarray(
                A_neg[d0:d0 + DI_SH].reshape(NDT, 128, 128), dtype=np.float32)
            im["dtb"] = np.ascontiguousarray(
                np.stack([mamba[m]["dt_proj_b"][d0:d0 + DI_SH]
                          .reshape(NDT, 128, 1) for m in range(3)]),
                dtype=np.float32)
            im["D_sh"] = np.ascontiguousarray(
                np.stack([mamba[m]["D"][d0:d0 + DI_SH]
                          .reshape(NDT, 128, 1) for m in range(3)]),
                dtype=np.float32)
            for m in range(3):
                mm = mamba[m]
                w1n, w2n = norm_fold[m]
                Win = mm["in_proj_w"] * w1n[None, :]
                Wsl = np.concatenate(
                    [Win[d0:d0 + DI_SH],
                     Win[D_INNER + d0:D_INNER + d0 + DI_SH]], axis=0)
                im[f"w_in_{m}"] = np.ascontiguousarray(
                    Wsl.T.reshape(4, 128, 1024), dtype=np.float16)
                im[f"w_conv_{m}"] = np.ascontiguousarray(
                    mm["conv_w"][d0:d0 + DI_SH].reshape(NDT, 128, D_CONV),
                    dtype=np.float32)
                im[f"w_xp_{m}"] = np.ascontiguousarray(
                    mm["x_proj_w"][:, d0:d0 + DI_SH].T.reshape(NDT, 128, 288),
                    dtype=np.float16)
                im[f"w_dt_{m}"] = np.ascontiguousarray(
                    mm["dt_proj_w"][d0:d0 + DI_SH].T, dtype=np.float16)
                im[f"w_out_{m}"] = np.ascontiguousarray(
                    mm["out_proj_w"][:, d0:d0 + DI_SH].T.reshape(NDT, 128, 512),
                    dtype=np.float16)
                gm = mlps[m]
                F1 = gm["fc1_w"] * w2n[None, :]
                im[f"w_fc1_{m}"] = np.ascontiguousarray(
                    F1.T.reshape(4, 128, 2 * HID_SMALL), dtype=np.float16)
                im[f"w_fc2_{m}"] = np.ascontiguousarray(
                    gm["fc2_w"].T.reshape(HID_SMALL // 128, 128, 512),
                    dtype=np.float16)
            tw = norms[TEXT_SEGS[r]]
            T1 = textp["fc1_w"] * tw[None, :]
            im["w_fc1_t"] = np.ascontiguousarray(
                T1.T.reshape(4, 128, 2 * HID_TEXT), dtype=np.float16)
            im["w_fc2_t"] = np.ascontiguousarray(
                textp["fc2_w"].T.reshape(HID_TEXT // 128, 128, 512),
                dtype=np.float16)
            in_maps.append(im)
    return in_maps


def assemble(results):
    out = np.zeros((2, OUT_TOTAL, D_MODEL), dtype=np.float32)
    for g in range(2):
        for r in range(R_SH):
            o = results[g * R_SH + r]["out"]
            out[g, r * 128:(r + 1) * 128] = o[0:128]
            out[g, 512 + r * 128:512 + (r + 1) * 128] = o[128:256]
            out[g, 1024 + r * 128:1024 + (r + 1) * 128] = o[256:384]
            out[g, 1536 + r * 128:1536 + (r + 1) * 128] = o[384:512]
            out[g, 2048 + r * 128:2048 + (r + 1) * 128] = o[512:640]
    return out


def kernel(x, T_motion, text_meta, params):
    assert int(T_motion) == T, f"kernel compiled for T_motion={T}"
    nc = _get_program()
    in_maps = prepare_in_maps(x, params)
    res = run_bass_kernel_spmd(nc, in_maps, core_ids=list(range(8)))
    return assemble(res.results)


# revision 23
# speedup vs baseline: 1.0050x; 1.0050x over previous
"""Trainium2 Bass kernel for nn_Block_Head_83047487635490 (Mamba motion block).

Sharding: 8 cores = 2 batch groups x 4-way tensor-parallel (d_inner 2048 -> 512
per core).  Per group: mamba in_proj/conv/x_proj/dt_proj/scan/out_proj sharded
over d_inner with a small fp16 all-reduce of x_db (288 x L) and a
reduce-scatter of out_proj partials over tokens; the gated MLPs and the text
MLP run token-parallel (full hidden) on the scattered quarters.

Selective scan: per (chunk, dtile, n-group) the recurrence
h = exp(delta*A_n) * h + (delta*u) * B_n runs as a single fp16
tensor_tensor_scan over Gn concatenated n-blocks (decay zeroed at block
starts resets the state); y = sum_n h*C_n via PE identity-matmul accumulation
in PSUM.
"""

import sys

if "/opt/trn_rl_repo" not in sys.path:
    sys.path.insert(0, "/opt/trn_rl_repo")

import numpy as np

from concourse import bacc, bass, mybir, tile
from concourse.bass_utils import run_bass_kernel_spmd

# ---------------------------------------------------------------- constants
B = 2
T = 512
N_TEXT = 128
TOTAL = 2048          # 3*T + 4*N_TEXT
D_MODEL = 512
D_STATE = 128
D_CONV = 4
D_INNER = 2048
DT_RANK = 32
HID_SMALL = 768
HID_TEXT = 2048
EPS = 1e-6
OUT_TOTAL = 2560      # music 512 + up 512 + down 1024 + text 512

R_SH = 4              # TP degree
DI_SH = D_INNER // R_SH
NDT = DI_SH // 128    # 4 d-tiles per core
F = 512               # time chunk
GN = 4                # n values per scan instruction
NG = D_STATE // GN    # 32
BLK = GN * F

BRANCHES = [(0, 512, 1), (512, 512, 1), (1024, 1024, 2)]  # (tok_off, L, chunks)
MLP_KEYS = ["gate_mlp_1", "gate_mlp_2", "gate_mlp_3"]
TEXT_SEGS = ["text_upper", "text_lower", "text_torso", "text_whole"]
MAMBA_KEYS = ["mamba_music", "mamba_up", "mamba_down"]

FP16 = mybir.dt.float16
FP32 = mybir.dt.float32
AF = mybir.ActivationFunctionType
ALU = mybir.AluOpType

OUT_SECS = [(0, 128), (128, 128), (256, 256), (512, 128)]
GROUPS = [[0, 1, 2, 3], [4, 5, 6, 7]]


def build_program():
    nc = bacc.Bacc("TRN2", target_bir_lowering=False, debug=False, num_devices=8)

    def din(name, shape, dt=FP16):
        return nc.dram_tensor(name, list(shape), dt, kind="ExternalInput").ap()

    x_t = din("x_t", (16, 128, 512))
    xq = din("xq", (640, 512), FP32)
    ident_in = din("ident", (128, 128))
    A_in = din("A_bc", (NDT, 128, 128), FP32)
    dtb_in = din("dtb", (3, NDT, 128, 1), FP32)
    D_in = din("D_sh", (3, NDT, 128, 1), FP32)
    w_in = [din(f"w_in_{m}", (4, 128, 1024)) for m in range(3)]
    w_conv = [din(f"w_conv_{m}", (NDT, 128, D_CONV), FP32) for m in range(3)]
    w_xp = [din(f"w_xp_{m}", (NDT, 128, 288)) for m in range(3)]
    w_dt = [din(f"w_dt_{m}", (DT_RANK, DI_SH)) for m in range(3)]
    w_out = [din(f"w_out_{m}", (NDT, 128, 512)) for m in range(3)]
    w_fc1 = [din(f"w_fc1_{m}", (4, 128, 2 * HID_SMALL)) for m in range(3)]
    w_fc2 = [din(f"w_fc2_{m}", (HID_SMALL // 128, 128, 512)) for m in range(3)]
    w_fc1_t = din("w_fc1_t", (4, 128, 2 * HID_TEXT))
    w_fc2_t = din("w_fc2_t", (HID_TEXT // 128, 128, 512))
    out_d = nc.dram_tensor("out", [640, 512], FP32, kind="ExternalOutput").ap()

    with tile.TileContext(nc) as tc:
        with (
            tc.tile_pool(name="const", bufs=1) as constp,
            tc.tile_pool(name="persist", bufs=1) as persist,
            tc.tile_pool(name="small", bufs=2) as small,
            tc.tile_pool(name="cols", bufs=4) as colsp,
            tc.tile_pool(name="wpool", bufs=4) as wpool,
            tc.tile_pool(name="wmlp", bufs=6) as wmlpp,
            tc.tile_pool(name="branch", bufs=1) as branchp,
            tc.tile_pool(name="scan", bufs=3) as scanp,
            tc.tile_pool(name="bc", bufs=3) as bcp,
            tc.tile_pool(name="post", bufs=2) as postp,
            tc.tile_pool(name="mlp", bufs=1) as mlpp,
            tc.tile_pool(name="hpool", bufs=16) as hpoolp,
            tc.tile_pool(name="psmm", bufs=4, space="PSUM") as psmm,
            tc.tile_pool(name="ypsum", bufs=1, space="PSUM") as ypsum,
            tc.tile_pool(name="dram", bufs=1, space="DRAM") as dramp,
        ):
            # ---------------- constants
            identT = constp.tile([128, 128], FP16, tag="ident")
            nc.sync.dma_start(out=identT[:], in_=ident_in[:])
            epscol = constp.tile([128, 1], FP32, tag="epscol")
            nc.vector.memset(epscol[:], EPS)
            A_t, dtb_t, D_t = [], {}, {}
            for dt in range(NDT):
                a = constp.tile([128, 128], FP32, tag=f"A{dt}")
                nc.sync.dma_start(out=a[:], in_=A_in[dt])
                A_t.append(a)
            for m in range(3):
                for dt in range(NDT):
                    bcol = constp.tile([128, 1], FP32, tag=f"dtb{m}_{dt}")
                    nc.sync.dma_start(out=bcol[:], in_=dtb_in[m, dt])
                    dtb_t[(m, dt)] = bcol
                    dcol = constp.tile([128, 1], FP32, tag=f"D{m}_{dt}")
                    nc.sync.dma_start(out=dcol[:], in_=D_in[m, dt])
                    D_t[(m, dt)] = dcol

            # ---------------- helpers
            def rms_tile(src_tile, dst_tile):
                scratch = small.tile([128, 512], FP16, tag="rms_scratch", bufs=1)
                ssq = colsp.tile([128, 1], FP32, tag="rms_ssq")
                nc.scalar.activation(out=scratch[:], in_=src_tile[:],
                                     func=AF.Square, accum_out=ssq[:])
                rr = colsp.tile([128, 1], FP32, tag="rms_rr")
                nc.scalar.activation(out=rr[:], in_=ssq[:], func=AF.Sqrt,
                                     scale=1.0 / D_MODEL, bias=epscol[:])
                inv = colsp.tile([128, 1], FP32, tag="rms_inv")
                nc.vector.reciprocal(out=inv[:], in_=rr[:])
                nc.vector.tensor_scalar(out=dst_tile[:], in0=src_tile[:],
                                        scalar1=inv[:], scalar2=None,
                                        op0=ALU.mult)

            def transpose_into(src_tile, n_blocks, put_block):
                for bb in range(n_blocks):
                    pst = psmm.tile([128, 128], FP16, tag="ps")
                    nc.tensor.transpose(pst[:], src_tile[:, bb * 128:(bb + 1) * 128],
                                        identT[:])
                    put_block(bb, pst)

            # ---------------- prolog: x -> rmsnorm -> transpose -> xnT
            xnT = [persist.tile([128, TOTAL], FP16, tag=f"xnT{k}",
                                name=f"xnT{k}") for k in range(4)]
            for i in range(16):
                xt = postp.tile([128, 512], FP16, tag="so", bufs=4, name="xt")
                nc.sync.dma_start(out=xt[:], in_=x_t[i])
                xn = postp.tile([128, 512], FP16, tag="y2", name="xn")
                rms_tile(xt, xn)

                def put(kt, pst, i=i):
                    nc.scalar.copy(out=xnT[kt][:, i * 128:(i + 1) * 128],
                                   in_=pst[:])
                transpose_into(xn, 4, put)

            # ---------------- generic gated-MLP block on one (128, 512) resid
            def mlp_block(resid_tile, w1_ap, hid, w2_ap, out_row):
                xn2 = mlpp.tile([128, 512], FP16, tag="mlp_xn")
                rms_tile(resid_tile, xn2)
                xn2T = []
                for kt in range(4):
                    dstt = mlpp.tile([128, 128], FP16, tag=f"mlp_xnT{kt}")
                    xn2T.append(dstt)

                def putx(kt, pst):
                    nc.scalar.copy(out=xn2T[kt][:], in_=pst[:])
                transpose_into(xn2, 4, putx)

                n_h = hid // 128
                hh = []
                for oy in range(n_h):
                    ps_y = psmm.tile([128, 128], FP32, tag="ps")
                    ps_g = psmm.tile([128, 128], FP32, tag="ps")
                    for half, ps in ((0, ps_y), (1, ps_g)):
                        o = oy + half * n_h
                        for kt in range(4):
                            w = wmlpp.tile([128, 128], FP16, tag="wm1")
                            nc.sync.dma_start(
                                out=w[:], in_=w1_ap[kt][:, o * 128:(o + 1) * 128])
                            nc.tensor.matmul(ps[:], w[:], xn2T[kt][:],
                                             start=(kt == 0), stop=(kt == 3))
                    sg = mlpp.tile([128, 128], FP16, tag="mlp_sg")
                    nc.scalar.activation(out=sg[:], in_=ps_g[:], func=AF.Sigmoid)
                    sg2 = mlpp.tile([128, 128], FP16, tag="mlp_sg2")
                    nc.vector.tensor_tensor(out=sg2[:], in0=ps_g[:], in1=sg[:],
                                            op=ALU.mult)
                    ht = hpoolp.tile([128, 128], FP16, tag="mlp_h")
                    nc.vector.tensor_tensor(out=ht[:], in0=ps_y[:], in1=sg2[:],
                                            op=ALU.mult)
                    hh.append(ht)
                mT = mlpp.tile([128, 512], FP16, tag="mlp_mT")
                for ot in range(4):
                    ps = psmm.tile([128, 128], FP32, tag="ps")
                    for kt in range(n_h):
                        w = wmlpp.tile([128, 128], FP16, tag="wm2")
                        nc.sync.dma_start(
                            out=w[:], in_=w2_ap[kt][:, ot * 128:(ot + 1) * 128])
                        nc.tensor.matmul(ps[:], w[:], hh[kt][:],
                                         start=(kt == 0), stop=(kt == n_h - 1))
                    mo = mlpp.tile([128, 128], FP16, tag="mlp_mo")
                    nc.scalar.copy(out=mo[:], in_=ps[:])
                    pst = psmm.tile([128, 128], FP16, tag="ps")
                    nc.tensor.transpose(pst[:], mo[:], identT[:])
                    nc.scalar.copy(out=mT[:, ot * 128:(ot + 1) * 128], in_=pst[:])
                fin = mlpp.tile([128, 512], FP32, tag="mlp_fin")
                nc.vector.tensor_tensor(out=fin[:], in0=mT[:], in1=resid_tile[:],
                                        op=ALU.add)
                nc.sync.dma_start(out=out_d[out_row:out_row + 128, :], in_=fin[:])

            # ---------------- per-branch state
            ar_out = {}
            u_tiles = {}
            du_tiles = {}
            delta_all = {}
            carry = {}
            rs_in = {}
            rs_out = {}
            for m in range(3):
                L = BRANCHES[m][1]
                rs_in[m] = dramp.tile([L, 512], FP16, tag=f"rsin{m}",
                                      name=f"rsin{m}")
                for ch in range(L // 512):
                    rs_out[(m, ch)] = dramp.tile(
                        [128, 512], FP16, tag=f"rsout{m}_{ch}",
                        name=f"rsout{m}_{ch}")

            # ================= pre-scan stage
            def pre_branch(m):
                tok_off, L, n_ch = BRANCHES[m]
                wi = []
                for kt in range(4):
                    wt = wpool.tile([128, 512], FP16, tag="w_in")
                    nc.sync.dma_start(out=wt[:], in_=w_in[m][kt][:, 0:512])
                    wi.append(wt)
                wxp = []
                for kt in range(NDT):
                    wt = wpool.tile([128, 288], FP16, tag="w_xp")
                    nc.sync.dma_start(out=wt[:], in_=w_xp[m][kt])
                    wxp.append(wt)
                wdt = wpool.tile([DT_RANK, DI_SH], FP16, tag="w_dt")
                nc.sync.dma_start(out=wdt[:], in_=w_dt[m][:])
                wcv = []
                for dt in range(NDT):
                    wt = wpool.tile([128, D_CONV], FP32, tag="w_conv")
                    nc.sync.dma_start(out=wt[:], in_=w_conv[m][dt])
                    wcv.append(wt)

                xiT = [branchp.tile([128, L + 3], FP16, tag=f"xiT{dt}_{m}",
                                    name=f"xiT{dt}_{m}") for dt in range(NDT)]
                uu = [xiT[dt][:, 3:3 + L] for dt in range(NDT)]
                duu = [branchp.tile([128, L], FP16, tag=f"du{dt}_{m}",
                                    name=f"du{dt}_{m}") for dt in range(NDT)]
                dall = branchp.tile([128, NDT * L], FP16, tag=f"delta_{m}",
                                    name=f"delta_{m}")

                for dt in range(NDT):
                    nc.vector.memset(xiT[dt][:, 0:3], 0.0)

                # in_proj (xi half only; z recomputed at post time)
                for ch in range(n_ch):
                    c0 = tok_off + ch * F
                    for ot in range(4):
                        ps = psmm.tile([128, 512], FP32, tag="ps")
                        for kt in range(4):
                            nc.tensor.matmul(ps[:], wi[kt][:, ot * 128:(ot + 1) * 128],
                                             xnT[kt][:, c0:c0 + F],
                                             start=(kt == 0), stop=(kt == 3))
                        nc.scalar.copy(
                            out=xiT[ot][:, 3 + ch * F:3 + (ch + 1) * F], in_=ps[:])

                # conv + silu -> u
                for dt in range(NDT):
                    acc0 = small.tile([128, 1024], FP16, tag="conv_a", bufs=1)
                    acc1 = small.tile([128, 1024], FP16, tag="conv_b", bufs=1)
                    nc.vector.tensor_scalar(out=acc0[:, 0:L], in0=xiT[dt][:, 0:L],
                                            scalar1=wcv[dt][:, 0:1], scalar2=None,
                                            op0=ALU.mult)
                    a, b_ = acc0, acc1
                    for j in range(1, D_CONV):
                        nc.vector.scalar_tensor_tensor(
                            out=b_[:, 0:L], in0=xiT[dt][:, j:j + L],
                            scalar=wcv[dt][:, j:j + 1], in1=a[:, 0:L],
                            op0=ALU.mult, op1=ALU.add)
                        a, b_ = b_, a
                    sgu = small.tile([128, 1024], FP16, tag="sgu", bufs=1)
                    nc.scalar.activation(out=sgu[:, 0:L], in_=a[:, 0:L],
                                         func=AF.Sigmoid)
                    nc.vector.tensor_tensor(out=uu[dt][:, 0:L], in0=a[:, 0:L],
                                            in1=sgu[:, 0:L], op=ALU.mult)

                # x_proj partials -> DRAM -> AllReduce (fp16)
                arin = dramp.tile([288, L], FP16, tag=f"arin{m}")
                arout = dramp.tile([288, L], FP16, tag=f"arout{m}")
                for ch in range(n_ch):
                    for po, pw in ((0, 128), (128, 128), (256, 32)):
                        ps = psmm.tile([128, 512], FP32, tag="ps")
                        for kt in range(NDT):
                            nc.tensor.matmul(
                                ps[:pw, :], wxp[kt][:, po:po + pw],
                                uu[kt][:, ch * F:(ch + 1) * F],
                                start=(kt == 0), stop=(kt == NDT - 1))
                        sb = small.tile([128, 512], FP16, tag="xdb_sb")
                        nc.scalar.copy(out=sb[:pw, :], in_=ps[:pw, :])
                        nc.sync.dma_start(
                            out=arin[po:po + pw, ch * F:(ch + 1) * F],
                            in_=sb[:pw, :])
                nc.gpsimd.collective_compute(
                    "AllReduce", ALU.add, replica_groups=GROUPS,
                    ins=[arin.opt()], outs=[arout.opt()])

                # dt_proj + softplus -> delta ; du = delta * u
                dtT = small.tile([DT_RANK, 1024], FP16, tag="dtT", bufs=1)
                nc.sync.dma_start(out=dtT[:, 0:L], in_=arout[0:DT_RANK, :])
                for dt in range(NDT):
                    for ch in range(n_ch):
                        ps = psmm.tile([128, 512], FP32, tag="ps")
                        nc.tensor.matmul(ps[:], wdt[:, dt * 128:(dt + 1) * 128],
                                         dtT[:, ch * F:(ch + 1) * F],
                                         start=True, stop=True)
                        spe = small.tile([128, 512], FP16, tag="spe", bufs=1)
                        nc.scalar.activation(out=spe[:], in_=ps[:], func=AF.Exp,
                                             bias=dtb_t[(m, dt)][:])
                        nc.scalar.activation(
                            out=dall[:, dt * L + ch * F: dt * L + (ch + 1) * F],
                            in_=spe[:], func=AF.Ln, bias=1.0)
                    nc.vector.tensor_tensor(out=duu[dt][:, 0:L],
                                            in0=dall[:, dt * L:dt * L + L],
                                            in1=uu[dt][:, 0:L], op=ALU.mult)

                ar_out[m] = arout
                u_tiles[m] = uu
                du_tiles[m] = duu
                delta_all[m] = dall
                if m == 2:
                    carry[m] = [branchp.tile([128, 128], FP16, tag=f"carry{dt}",
                                             name=f"carry{dt}")
                                for dt in range(NDT)]

            # ================= scan + post of one chunk
            def scan_chunk(m, ch):
                tok_off, L, n_ch = BRANCHES[m]
                arout = ar_out[m]
                dall = delta_all[m]
                duu = du_tiles[m]
                chained = (m == 2 and ch == 1)
                save_carry = (m == 2 and ch == 0)

                y_ps = [ypsum.tile([128, 512], FP32, tag=f"yps{dt}",
                                   name=f"yps{dt}_{m}_{ch}")
                        for dt in range(NDT)]
                for ng in range(NG):
                    n0 = ng * GN
                    b_bc = bcp.tile([128, BLK], FP16, tag="b_bc")
                    src = arout[DT_RANK + n0:DT_RANK + n0 + GN,
                                ch * F:(ch + 1) * F]
                    nc.sync.dma_start(
                        out=b_bc[:].rearrange("p (g f) -> p g f", g=GN),
                        in_=src.unsqueeze(0).broadcast_to([128, GN, F]))
                    c_bc = bcp.tile([128, BLK], FP16, tag="c_bc")
                    src = arout[DT_RANK + D_STATE + n0:
                                DT_RANK + D_STATE + n0 + GN,
                                ch * F:(ch + 1) * F]
                    nc.sync.dma_start(
                        out=c_bc[:].rearrange("p (g f) -> p g f", g=GN),
                        in_=src.unsqueeze(0).broadcast_to([128, GN, F]))

                    for dt in range(NDT):
                        dA = scanp.tile([128, BLK], FP16, tag="dA", bufs=2)
                        dA_g = dA[:].rearrange("p (g f) -> p g f", g=GN)
                        for j in range(GN):
                            nc.scalar.activation(
                                out=dA_g[:, j, :],
                                in_=dall[:, dt * L + ch * F:
                                         dt * L + (ch + 1) * F],
                                func=AF.Exp,
                                scale=A_t[dt][:, n0 + j:n0 + j + 1])
                        bb = scanp.tile([128, BLK], FP16, tag="bb")
                        du_view = duu[dt][:, ch * F:(ch + 1) * F]
                        nc.vector.tensor_tensor(
                            out=bb[:].rearrange("p (g f) -> p g f", g=GN),
                            in0=du_view.unsqueeze(1).broadcast_to([128, GN, F]),
                            in1=b_bc[:].rearrange("p (g f) -> p g f", g=GN),
                            op=ALU.mult)
                        if chained:
                            tmp = colsp.tile([128, GN], FP16, tag="fix")
                            nc.vector.tensor_tensor(
                                out=tmp[:], in0=dA_g[:, :, 0],
                                in1=carry[2][dt][:, n0:n0 + GN], op=ALU.mult)
                            bst = bb[:].rearrange("p (g f) -> p g f", g=GN)[:, :, 0]
                            nc.vector.tensor_tensor(out=bst, in0=tmp[:],
                                                    in1=bst, op=ALU.add)
                        nc.vector.memset(dA_g[:, :, 0], 0.0)
                        hh = scanp.tile([128, BLK], FP16, tag="hh")
                        nc.vector.tensor_tensor_scan(
                            out=hh[:], data0=dA[:], data1=bb[:], initial=0.0,
                            op0=ALU.mult, op1=ALU.add)
                        if save_carry:
                            nc.vector.tensor_copy(
                                out=carry[2][dt][:, n0:n0 + GN],
                                in_=hh[:].rearrange("p (g f) -> p g f",
                                                    g=GN)[:, :, F - 1])
                        prod = scanp.tile([128, BLK], FP16, tag="prod")
                        nc.vector.tensor_tensor(
                            out=prod[:], in0=hh[:], in1=c_bc[:], op=ALU.mult)
                        for g in range(GN):
                            nc.tensor.matmul(
                                y_ps[dt][:], identT[:],
                                prod[:, g * F:(g + 1) * F],
                                start=(ng == 0 and g == 0),
                                stop=(ng == NG - 1 and g == GN - 1))

                # ---- post: z -> silu ; y3 = (y + D*u) * silu(z) ; out_proj
                c0 = tok_off + ch * F
                wiz = []
                for kt in range(4):
                    wt = wpool.tile([128, 512], FP16, tag="w_z")
                    nc.sync.dma_start(out=wt[:], in_=w_in[m][kt][:, 512:1024])
                    wiz.append(wt)
                y3s = []
                for dt in range(NDT):
                    psz = psmm.tile([128, 512], FP32, tag="ps")
                    for kt in range(4):
                        nc.tensor.matmul(psz[:], wiz[kt][:, dt * 128:(dt + 1) * 128],
                                         xnT[kt][:, c0:c0 + F],
                                         start=(kt == 0), stop=(kt == 3))
                    sgz = postp.tile([128, 512], FP16, tag="sgz")
                    nc.scalar.activation(out=sgz[:], in_=psz[:], func=AF.Sigmoid)
                    szt = postp.tile([128, 512], FP16, tag="sz")
                    nc.vector.tensor_tensor(out=szt[:], in0=psz[:], in1=sgz[:],
                                            op=ALU.mult)
                    y2 = postp.tile([128, 512], FP16, tag="y2")
                    nc.vector.scalar_tensor_tensor(
                        out=y2[:], in0=u_tiles[m][dt][:, ch * F:(ch + 1) * F],
                        scalar=D_t[(m, dt)][:], in1=y_ps[dt][:],
                        op0=ALU.mult, op1=ALU.add)
                    y3 = postp.tile([128, 512], FP16, tag="y3", bufs=4)
                    nc.vector.tensor_tensor(out=y3[:], in0=y2[:], in1=szt[:],
                                            op=ALU.mult)
                    y3s.append(y3)

                wo = []
                for kt in range(NDT):
                    wt = wpool.tile([128, 512], FP16, tag="w_out")
                    nc.sync.dma_start(out=wt[:], in_=w_out[m][kt])
                    wo.append(wt)
                so = []
                for ot in range(4):
                    ps = psmm.tile([128, 512], FP32, tag="ps")
                    for kt in range(NDT):
                        nc.tensor.matmul(ps[:], wo[kt][:, ot * 128:(ot + 1) * 128],
                                         y3s[kt][:], start=(kt == 0),
                                         stop=(kt == NDT - 1))
                    st = postp.tile([128, 512], FP16, tag="so", bufs=4)
                    nc.scalar.copy(out=st[:], in_=ps[:])
                    so.append(st)
                for tt in range(4):
                    ob = postp.tile([128, 512], FP16, tag="obuf")
                    for ot in range(4):
                        pst = psmm.tile([128, 128], FP16, tag="ps")
                        nc.tensor.transpose(pst[:],
                                            so[ot][:, tt * 128:(tt + 1) * 128],
                                            identT[:])
                        nc.scalar.copy(out=ob[:, ot * 128:(ot + 1) * 128],
                                       in_=pst[:])
                    nc.sync.dma_start(
                        out=rs_in[m][ch * F + tt * 128:ch * F + (tt + 1) * 128, :],
                        in_=ob[:])

            # ================= post-RS MLP (one 512-token chunk of branch m)
            def mlp_branch(m, ch=0):
                nc.gpsimd.collective_compute(
                    "ReduceScatter", ALU.add, replica_groups=GROUPS,
                    ins=[rs_in[m][ch * 512:(ch + 1) * 512, :]],
                    outs=[rs_out[(m, ch)].opt()])
                sec0, nrow = OUT_SECS[m]
                rsv = mlpp.tile([128, 512], FP16, tag="rsv")
                nc.sync.dma_start(out=rsv[:], in_=rs_out[(m, ch)][:])
                xqv = mlpp.tile([128, 512], FP32, tag="xqv")
                nc.sync.dma_start(
                    out=xqv[:],
                    in_=xq[sec0 + ch * 128:sec0 + (ch + 1) * 128, :])
                resid = mlpp.tile([128, 512], FP32, tag="resid")
                nc.vector.tensor_tensor(out=resid[:], in0=rsv[:], in1=xqv[:],
                                        op=ALU.add)
                mlp_block(resid, w_fc1[m], HID_SMALL, w_fc2[m],
                          sec0 + ch * 128)

            # ---------------- schedule (program order guides the Tile
            # scheduler: each branch's AllReduce flies while the previous
            # branch's scan keeps the Vector engine saturated)
            pre_branch(0)
            xq_text = mlpp.tile([128, 512], FP32, tag="xq_text")
            nc.sync.dma_start(out=xq_text[:], in_=xq[512:640, :])
            mlp_block(xq_text, w_fc1_t, HID_TEXT, w_fc2_t, 512)
            pre_branch(1)
            scan_chunk(0, 0)
            pre_branch(2)
            scan_chunk(1, 0)
            mlp_branch(0)
            scan_chunk(2, 0)
            mlp_branch(1)
            mlp_branch(2, 0)
            scan_chunk(2, 1)
            mlp_branch(2, 1)

    nc.finalize()
    return nc


_PROGRAM = None


def _get_program():
    global _PROGRAM
    if _PROGRAM is None:
        _PROGRAM = build_program()
    return _PROGRAM


def _np(a, dt=np.float32):
    return np.ascontiguousarray(np.asarray(a), dtype=dt)


def prepare_in_maps(x, params):
    x = _np(x)
    norms = {k: _np(v) for k, v in params["norms"].items()}
    mamba = [{k: _np(v) for k, v in params[mk].items()} for mk in MAMBA_KEYS]
    mlps = [{k: _np(v) for k, v in params[mk].items()} for mk in MLP_KEYS]
    textp = {k: _np(v) for k, v in params["text_mlp"].items()}
    ident = np.eye(128, dtype=np.float16)
    norm_fold = [
        (norms["music_1"], norms["music_2"]),
        (norms["up_1"], norms["up_2"]),
        (norms["down_1"], norms["down_2"]),
    ]
    A_neg = -np.exp(mamba[0]["A_log"])
    for mmx in mamba[1:]:
        assert np.allclose(-np.exp(mmx["A_log"]), A_neg), \
            "mamba A_log differ across branches; not supported"
    for mm in mamba:
        assert np.all(mm["conv_b"] == 0), "conv bias not folded"
    for gm in mlps + [textp]:
        assert np.all(gm["fc1_b"] == 0) and np.all(gm["fc2_b"] == 0)

    in_maps = []
    for g in range(2):
        xg = x[g]
        x_t16 = np.ascontiguousarray(
            xg.astype(np.float16).reshape(16, 128, 512))
        for r in range(R_SH):
            d0 = r * DI_SH
            im = {"x_t": x_t16, "ident": ident}
            im["xq"] = np.ascontiguousarray(np.concatenate([
                xg[r * 128:(r + 1) * 128],
                xg[512 + r * 128:512 + (r + 1) * 128],
                xg[1024 + r * 128:1024 + (r + 1) * 128],
                xg[1536 + r * 128:1536 + (r + 1) * 128],
                xg[1536 + r * 128:1536 + (r + 1) * 128],
            ], axis=0), dtype=np.float32)
            im["A_bc"] = np.ascontiguousarray(
                A_neg[d0:d0 + DI_SH].reshape(NDT, 128, 128), dtype=np.float32)
            im["dtb"] = np.ascontiguousarray(
                np.stack([mamba[m]["dt_proj_b"][d0:d0 + DI_SH]
                          .reshape(NDT, 128, 1) for m in range(3)]),
                dtype=np.float32)
            im["D_sh"] = np.ascontiguousarray(
                np.stack([mamba[m]["D"][d0:d0 + DI_SH]
                          .reshape(NDT, 128, 1) for m in range(3)]),
                dtype=np.float32)
            for m in range(3):
                mm = mamba[m]
                w1n, w2n = norm_fold[m]
                Win = mm["in_proj_w"] * w1n[None, :]
                Wsl = np.concatenate(
                    [Win[d0:d0 + DI_SH],
                     Win[D_INNER + d0:D_INNER + d0 + DI_SH]], axis=0)
                im[f"w_in_{m}"] = np.ascontiguousarray(
                    Wsl.T.reshape(4, 128, 1024), dtype=np.float16)
                im[f"w_conv_{m}"] = np.ascontiguousarray(
                    mm["conv_w"][d0:d0 + DI_SH].reshape(NDT, 128, D_CONV),
                    dtype=np.float32)
                im[f"w_xp_{m}"] = np.ascontiguousarray(
                    mm["x_proj_w"][:, d0:d0 + DI_SH].T.reshape(NDT, 128, 288),
                    dtype=np.float16)
                im[f"w_dt_{m}"] = np.ascontiguousarray(
                    mm["dt_proj_w"][d0:d0 + DI_SH].T, dtype=np.float16)
                im[f"w_out_{m}"] = np.ascontiguousarray(
                    mm["out_proj_w"][:, d0:d0 + DI_SH].T.reshape(NDT, 128, 512),
                    dtype=np.float16)
                gm = mlps[m]
                F1 = gm["fc1_w"] * w2n[None, :]
                im[f"w_fc1_{m}"] = np.ascontiguousarray(
                    F1.T.reshape(4, 128, 2 * HID_SMALL), dtype=np.float16)
                im[f"w_fc2_{m}"] = np.ascontiguousarray(
                    gm["fc2_w"].T.reshape(HID_SMALL // 128, 128, 512),
                    dtype=np.float16)
            tw = norms[TEXT_SEGS[r]]
            T1 = textp["fc1_w"] * tw[None, :]
            im["w_fc1_t"] = np.ascontiguousarray(
                T1.T.reshape(4, 128, 2 * HID_TEXT), dtype=np.float16)
            im["w_fc2_t"] = np.ascontiguousarray(
                textp["fc2_w"].T.reshape(HID_TEXT // 128, 128, 512),
                dtype=np.float16)
            in_maps.append(im)
    return in_maps


def assemble(results):
    out = np.zeros((2, OUT_TOTAL, D_MODEL), dtype=np.float32)
    for g in range(2):
        for r in range(R_SH):
            o = results[g * R_SH + r]["out"]
            out[g, r * 128:(r + 1) * 128] = o[0:128]
            out[g, 512 + r * 128:512 + (r + 1) * 128] = o[128:256]
            out[g, 1024 + r * 128:1024 + (r + 1) * 128] = o[256:384]
            out[g, 1536 + r * 128:1536 + (r + 1) * 128] = o[384:512]
            out[g, 2048 + r * 128:2048 + (r + 1) * 128] = o[512:640]
    return out


def kernel(x, T_motion, text_meta, params):
    assert int(T_motion) == T, f"kernel compiled for T_motion={T}"
    nc = _get_program()
    in_maps = prepare_in_maps(x, params)
    res = run_bass_kernel_spmd(nc, in_maps, core_ids=list(range(8)))
    return assemble(res.results)


# revision 26
# speedup vs baseline: 1.0118x; 1.0067x over previous
"""Trainium2 Bass kernel for nn_Block_Head_83047487635490 (Mamba motion block).

Sharding: 8 cores = 2 batch groups x 4-way tensor-parallel (d_inner 2048 -> 512
per core).  Per group: mamba in_proj/conv/x_proj/dt_proj/scan/out_proj sharded
over d_inner with a small fp16 all-reduce of x_db (288 x L) and a
reduce-scatter of out_proj partials over tokens; the gated MLPs and the text
MLP run token-parallel (full hidden) on the scattered quarters.

Selective scan: per (chunk, dtile, n-group) the recurrence
h = exp(delta*A_n) * h + (delta*u) * B_n runs as a single fp16
tensor_tensor_scan over Gn concatenated n-blocks (decay zeroed at block
starts resets the state); y = sum_n h*C_n via PE identity-matmul accumulation
in PSUM.
"""

import sys

if "/opt/trn_rl_repo" not in sys.path:
    sys.path.insert(0, "/opt/trn_rl_repo")

import numpy as np

from concourse import bacc, bass, mybir, tile
from concourse.bass_utils import run_bass_kernel_spmd

# ---------------------------------------------------------------- constants
B = 2
T = 512
N_TEXT = 128
TOTAL = 2048          # 3*T + 4*N_TEXT
D_MODEL = 512
D_STATE = 128
D_CONV = 4
D_INNER = 2048
DT_RANK = 32
HID_SMALL = 768
HID_TEXT = 2048
EPS = 1e-6
OUT_TOTAL = 2560      # music 512 + up 512 + down 1024 + text 512

R_SH = 4              # TP degree
DI_SH = D_INNER // R_SH
NDT = DI_SH // 128    # 4 d-tiles per core
F = 512               # time chunk
GN = 4                # n values per scan instruction
NG = D_STATE // GN    # 32
BLK = GN * F

BRANCHES = [(0, 512, 1), (512, 512, 1), (1024, 1024, 2)]  # (tok_off, L, chunks)
MLP_KEYS = ["gate_mlp_1", "gate_mlp_2", "gate_mlp_3"]
TEXT_SEGS = ["text_upper", "text_lower", "text_torso", "text_whole"]
MAMBA_KEYS = ["mamba_music", "mamba_up", "mamba_down"]

FP16 = mybir.dt.float16
FP32 = mybir.dt.float32
AF = mybir.ActivationFunctionType
ALU = mybir.AluOpType

OUT_SECS = [(0, 128), (128, 128), (256, 256), (512, 128)]
GROUPS = [[0, 1, 2, 3], [4, 5, 6, 7]]


def build_program():
    nc = bacc.Bacc("TRN2", target_bir_lowering=False, debug=False, num_devices=8)

    def din(name, shape, dt=FP16):
        return nc.dram_tensor(name, list(shape), dt, kind="ExternalInput").ap()

    x_t = din("x_t", (16, 128, 512))
    xq = din("xq", (640, 512), FP32)
    ident_in = din("ident", (128, 128))
    A_in = din("A_bc", (NDT, 128, 128), FP32)
    dtb_in = din("dtb", (3, NDT, 128, 1), FP32)
    D_in = din("D_sh", (3, NDT, 128, 1), FP32)
    w_in = [din(f"w_in_{m}", (4, 128, 1024)) for m in range(3)]
    w_conv = [din(f"w_conv_{m}", (NDT, 128, D_CONV), FP32) for m in range(3)]
    w_xp = [din(f"w_xp_{m}", (NDT, 128, 288)) for m in range(3)]
    w_dt = [din(f"w_dt_{m}", (DT_RANK, DI_SH)) for m in range(3)]
    w_out = [din(f"w_out_{m}", (NDT, 128, 512)) for m in range(3)]
    w_fc1 = [din(f"w_fc1_{m}", (4, 128, 2 * HID_SMALL)) for m in range(3)]
    w_fc2 = [din(f"w_fc2_{m}", (HID_SMALL // 128, 128, 512)) for m in range(3)]
    w_fc1_t = din("w_fc1_t", (4, 128, 2 * HID_TEXT))
    w_fc2_t = din("w_fc2_t", (HID_TEXT // 128, 128, 512))
    out_d = nc.dram_tensor("out", [640, 512], FP32, kind="ExternalOutput").ap()

    with tile.TileContext(nc) as tc:
        with (
            tc.tile_pool(name="const", bufs=1) as constp,
            tc.tile_pool(name="persist", bufs=1) as persist,
            tc.tile_pool(name="small", bufs=2) as small,
            tc.tile_pool(name="cols", bufs=4) as colsp,
            tc.tile_pool(name="wpool", bufs=4) as wpool,
            tc.tile_pool(name="wmlp", bufs=4) as wmlpp,
            tc.tile_pool(name="branch", bufs=1) as branchp,
            tc.tile_pool(name="scan", bufs=3) as scanp,
            tc.tile_pool(name="bc", bufs=4) as bcp,
            tc.tile_pool(name="post", bufs=2) as postp,
            tc.tile_pool(name="mlp", bufs=1) as mlpp,
            tc.tile_pool(name="hpool", bufs=16) as hpoolp,
            tc.tile_pool(name="psmm", bufs=4, space="PSUM") as psmm,
            tc.tile_pool(name="ypsum", bufs=1, space="PSUM") as ypsum,
            tc.tile_pool(name="dram", bufs=1, space="DRAM") as dramp,
        ):
            # ---------------- constants
            identT = constp.tile([128, 128], FP16, tag="ident")
            nc.sync.dma_start(out=identT[:], in_=ident_in[:])
            epscol = constp.tile([128, 1], FP32, tag="epscol")
            nc.vector.memset(epscol[:], EPS)
            A_t, dtb_t, D_t = [], {}, {}
            for dt in range(NDT):
                a = constp.tile([128, 128], FP32, tag=f"A{dt}")
                nc.sync.dma_start(out=a[:], in_=A_in[dt])
                A_t.append(a)
            for m in range(3):
                for dt in range(NDT):
                    bcol = constp.tile([128, 1], FP32, tag=f"dtb{m}_{dt}")
                    nc.sync.dma_start(out=bcol[:], in_=dtb_in[m, dt])
                    dtb_t[(m, dt)] = bcol
                    dcol = constp.tile([128, 1], FP32, tag=f"D{m}_{dt}")
                    nc.sync.dma_start(out=dcol[:], in_=D_in[m, dt])
                    D_t[(m, dt)] = dcol

            # ---------------- helpers
            def rms_tile(src_tile, dst_tile):
                scratch = small.tile([128, 512], FP16, tag="rms_scratch", bufs=1)
                ssq = colsp.tile([128, 1], FP32, tag="rms_ssq")
                nc.scalar.activation(out=scratch[:], in_=src_tile[:],
                                     func=AF.Square, accum_out=ssq[:])
                rr = colsp.tile([128, 1], FP32, tag="rms_rr")
                nc.scalar.activation(out=rr[:], in_=ssq[:], func=AF.Sqrt,
                                     scale=1.0 / D_MODEL, bias=epscol[:])
                inv = colsp.tile([128, 1], FP32, tag="rms_inv")
                nc.vector.reciprocal(out=inv[:], in_=rr[:])
                nc.vector.tensor_scalar(out=dst_tile[:], in0=src_tile[:],
                                        scalar1=inv[:], scalar2=None,
                                        op0=ALU.mult)

            def transpose_into(src_tile, n_blocks, put_block):
                for bb in range(n_blocks):
                    pst = psmm.tile([128, 128], FP16, tag="ps")
                    nc.tensor.transpose(pst[:], src_tile[:, bb * 128:(bb + 1) * 128],
                                        identT[:])
                    put_block(bb, pst)

            # ---------------- prolog: x -> rmsnorm -> transpose -> xnT
            xnT = [persist.tile([128, TOTAL], FP16, tag=f"xnT{k}",
                                name=f"xnT{k}") for k in range(4)]
            for i in range(16):
                xt = postp.tile([128, 512], FP16, tag="so", bufs=4, name="xt")
                nc.sync.dma_start(out=xt[:], in_=x_t[i])
                xn = postp.tile([128, 512], FP16, tag="y2", name="xn")
                rms_tile(xt, xn)

                def put(kt, pst, i=i):
                    nc.scalar.copy(out=xnT[kt][:, i * 128:(i + 1) * 128],
                                   in_=pst[:])
                transpose_into(xn, 4, put)

            # ---------------- generic gated-MLP block on one (128, 512) resid
            def mlp_block(resid_tile, w1_ap, hid, w2_ap, out_row):
                xn2 = mlpp.tile([128, 512], FP16, tag="mlp_xn")
                rms_tile(resid_tile, xn2)
                xn2T = []
                for kt in range(4):
                    dstt = mlpp.tile([128, 128], FP16, tag=f"mlp_xnT{kt}")
                    xn2T.append(dstt)

                def putx(kt, pst):
                    nc.scalar.copy(out=xn2T[kt][:], in_=pst[:])
                transpose_into(xn2, 4, putx)

                n_h = hid // 128
                hh = []
                for oy in range(n_h):
                    ps_y = psmm.tile([128, 128], FP32, tag="ps")
                    ps_g = psmm.tile([128, 128], FP32, tag="ps")
                    for half, ps in ((0, ps_y), (1, ps_g)):
                        o = oy + half * n_h
                        for kt in range(4):
                            w = wmlpp.tile([128, 128], FP16, tag="wm1")
                            nc.sync.dma_start(
                                out=w[:], in_=w1_ap[kt][:, o * 128:(o + 1) * 128])
                            nc.tensor.matmul(ps[:], w[:], xn2T[kt][:],
                                             start=(kt == 0), stop=(kt == 3))
                    sg = mlpp.tile([128, 128], FP16, tag="mlp_sg")
                    nc.scalar.activation(out=sg[:], in_=ps_g[:], func=AF.Sigmoid)
                    sg2 = mlpp.tile([128, 128], FP16, tag="mlp_sg2")
                    nc.vector.tensor_tensor(out=sg2[:], in0=ps_g[:], in1=sg[:],
                                            op=ALU.mult)
                    ht = hpoolp.tile([128, 128], FP16, tag="mlp_h")
                    nc.vector.tensor_tensor(out=ht[:], in0=ps_y[:], in1=sg2[:],
                                            op=ALU.mult)
                    hh.append(ht)
                mT = mlpp.tile([128, 512], FP16, tag="mlp_mT")
                for ot in range(4):
                    ps = psmm.tile([128, 128], FP32, tag="ps")
                    for kt in range(n_h):
                        w = wmlpp.tile([128, 128], FP16, tag="wm2")
                        nc.sync.dma_start(
                            out=w[:], in_=w2_ap[kt][:, ot * 128:(ot + 1) * 128])
                        nc.tensor.matmul(ps[:], w[:], hh[kt][:],
                                         start=(kt == 0), stop=(kt == n_h - 1))
                    mo = mlpp.tile([128, 128], FP16, tag="mlp_mo")
                    nc.scalar.copy(out=mo[:], in_=ps[:])
                    pst = psmm.tile([128, 128], FP16, tag="ps")
                    nc.tensor.transpose(pst[:], mo[:], identT[:])
                    nc.scalar.copy(out=mT[:, ot * 128:(ot + 1) * 128], in_=pst[:])
                fin = mlpp.tile([128, 512], FP32, tag="mlp_fin")
                nc.vector.tensor_tensor(out=fin[:], in0=mT[:], in1=resid_tile[:],
                                        op=ALU.add)
                nc.sync.dma_start(out=out_d[out_row:out_row + 128, :], in_=fin[:])

            # ---------------- per-branch state
            ar_out = {}
            u_tiles = {}
            du_tiles = {}
            delta_all = {}
            carry = {}
            rs_in = {}
            rs_out = {}
            for m in range(3):
                L = BRANCHES[m][1]
                rs_in[m] = dramp.tile([L, 512], FP16, tag=f"rsin{m}",
                                      name=f"rsin{m}")
                for ch in range(L // 512):
                    rs_out[(m, ch)] = dramp.tile(
                        [128, 512], FP16, tag=f"rsout{m}_{ch}",
                        name=f"rsout{m}_{ch}")

            # ================= pre-scan stage
            def pre_branch(m):
                tok_off, L, n_ch = BRANCHES[m]
                wi = []
                for kt in range(4):
                    wt = wpool.tile([128, 512], FP16, tag="w_in")
                    nc.sync.dma_start(out=wt[:], in_=w_in[m][kt][:, 0:512])
                    wi.append(wt)
                wxp = []
                for kt in range(NDT):
                    wt = wpool.tile([128, 288], FP16, tag="w_xp")
                    nc.sync.dma_start(out=wt[:], in_=w_xp[m][kt])
                    wxp.append(wt)
                wdt = wpool.tile([DT_RANK, DI_SH], FP16, tag="w_dt")
                nc.sync.dma_start(out=wdt[:], in_=w_dt[m][:])
                wcv = []
                for dt in range(NDT):
                    wt = wpool.tile([128, D_CONV], FP32, tag="w_conv")
                    nc.sync.dma_start(out=wt[:], in_=w_conv[m][dt])
                    wcv.append(wt)

                xiT = [branchp.tile([128, L + 3], FP16, tag=f"xiT{dt}_{m}",
                                    name=f"xiT{dt}_{m}") for dt in range(NDT)]
                uu = [xiT[dt][:, 3:3 + L] for dt in range(NDT)]
                duu = [branchp.tile([128, L], FP16, tag=f"du{dt}_{m}",
                                    name=f"du{dt}_{m}") for dt in range(NDT)]
                dall = branchp.tile([128, NDT * L], FP16, tag=f"delta_{m}",
                                    name=f"delta_{m}")

                for dt in range(NDT):
                    nc.vector.memset(xiT[dt][:, 0:3], 0.0)

                # in_proj (xi half only; z recomputed at post time)
                for ch in range(n_ch):
                    c0 = tok_off + ch * F
                    for ot in range(4):
                        ps = psmm.tile([128, 512], FP32, tag="ps")
                        for kt in range(4):
                            nc.tensor.matmul(ps[:], wi[kt][:, ot * 128:(ot + 1) * 128],
                                             xnT[kt][:, c0:c0 + F],
                                             start=(kt == 0), stop=(kt == 3))
                        nc.scalar.copy(
                            out=xiT[ot][:, 3 + ch * F:3 + (ch + 1) * F], in_=ps[:])

                # conv + silu -> u
                for dt in range(NDT):
                    acc0 = small.tile([128, 1024], FP16, tag="conv_a", bufs=1)
                    acc1 = small.tile([128, 1024], FP16, tag="conv_b", bufs=1)
                    nc.vector.tensor_scalar(out=acc0[:, 0:L], in0=xiT[dt][:, 0:L],
                                            scalar1=wcv[dt][:, 0:1], scalar2=None,
                                            op0=ALU.mult)
                    a, b_ = acc0, acc1
                    for j in range(1, D_CONV):
                        nc.vector.scalar_tensor_tensor(
                            out=b_[:, 0:L], in0=xiT[dt][:, j:j + L],
                            scalar=wcv[dt][:, j:j + 1], in1=a[:, 0:L],
                            op0=ALU.mult, op1=ALU.add)
                        a, b_ = b_, a
                    nc.scalar.activation(out=b_[:, 0:L], in_=a[:, 0:L],
                                         func=AF.Sigmoid)
                    nc.vector.tensor_tensor(out=uu[dt][:, 0:L], in0=a[:, 0:L],
                                            in1=b_[:, 0:L], op=ALU.mult)

                # x_proj partials -> DRAM -> AllReduce (fp16)
                arin = dramp.tile([288, L], FP16, tag=f"arin{m}")
                arout = dramp.tile([288, L], FP16, tag=f"arout{m}")
                for ch in range(n_ch):
                    for po, pw in ((0, 128), (128, 128), (256, 32)):
                        ps = psmm.tile([128, 512], FP32, tag="ps")
                        for kt in range(NDT):
                            nc.tensor.matmul(
                                ps[:pw, :], wxp[kt][:, po:po + pw],
                                uu[kt][:, ch * F:(ch + 1) * F],
                                start=(kt == 0), stop=(kt == NDT - 1))
                        sb = small.tile([128, 512], FP16, tag="xdb_sb")
                        nc.scalar.copy(out=sb[:pw, :], in_=ps[:pw, :])
                        nc.sync.dma_start(
                            out=arin[po:po + pw, ch * F:(ch + 1) * F],
                            in_=sb[:pw, :])
                nc.gpsimd.collective_compute(
                    "AllReduce", ALU.add, replica_groups=GROUPS,
                    ins=[arin.opt()], outs=[arout.opt()])

                # dt_proj + softplus -> delta ; du = delta * u
                dtT = small.tile([DT_RANK, 1024], FP16, tag="dtT", bufs=1)
                nc.sync.dma_start(out=dtT[:, 0:L], in_=arout[0:DT_RANK, :])
                for dt in range(NDT):
                    for ch in range(n_ch):
                        ps = psmm.tile([128, 512], FP32, tag="ps")
                        nc.tensor.matmul(ps[:], wdt[:, dt * 128:(dt + 1) * 128],
                                         dtT[:, ch * F:(ch + 1) * F],
                                         start=True, stop=True)
                        spe = small.tile([128, 512], FP16, tag="spe", bufs=1)
                        nc.scalar.activation(out=spe[:], in_=ps[:], func=AF.Exp,
                                             bias=dtb_t[(m, dt)][:])
                        nc.scalar.activation(
                            out=dall[:, dt * L + ch * F: dt * L + (ch + 1) * F],
                            in_=spe[:], func=AF.Ln, bias=1.0)
                    nc.vector.tensor_tensor(out=duu[dt][:, 0:L],
                                            in0=dall[:, dt * L:dt * L + L],
                                            in1=uu[dt][:, 0:L], op=ALU.mult)

                ar_out[m] = arout
                u_tiles[m] = uu
                du_tiles[m] = duu
                delta_all[m] = dall
                if m == 2:
                    carry[m] = [branchp.tile([128, 128], FP16, tag=f"carry{dt}",
                                             name=f"carry{dt}")
                                for dt in range(NDT)]

            # ================= scan + post of one chunk
            def scan_chunk(m, ch):
                tok_off, L, n_ch = BRANCHES[m]
                arout = ar_out[m]
                dall = delta_all[m]
                duu = du_tiles[m]
                chained = (m == 2 and ch == 1)
                save_carry = (m == 2 and ch == 0)

                y_ps = [ypsum.tile([128, 512], FP32, tag=f"yps{dt}",
                                   name=f"yps{dt}_{m}_{ch}")
                        for dt in range(NDT)]
                for ng in range(NG):
                    n0 = ng * GN
                    b_bc = bcp.tile([128, BLK], FP16, tag="b_bc")
                    src = arout[DT_RANK + n0:DT_RANK + n0 + GN,
                                ch * F:(ch + 1) * F]
                    nc.sync.dma_start(
                        out=b_bc[:].rearrange("p (g f) -> p g f", g=GN),
                        in_=src.unsqueeze(0).broadcast_to([128, GN, F]))
                    c_bc = bcp.tile([128, BLK], FP16, tag="c_bc")
                    src = arout[DT_RANK + D_STATE + n0:
                                DT_RANK + D_STATE + n0 + GN,
                                ch * F:(ch + 1) * F]
                    nc.sync.dma_start(
                        out=c_bc[:].rearrange("p (g f) -> p g f", g=GN),
                        in_=src.unsqueeze(0).broadcast_to([128, GN, F]))

                    for dt in range(NDT):
                        dA = scanp.tile([128, BLK], FP16, tag="dA", bufs=2)
                        dA_g = dA[:].rearrange("p (g f) -> p g f", g=GN)
                        for j in range(GN):
                            nc.scalar.activation(
                                out=dA_g[:, j, :],
                                in_=dall[:, dt * L + ch * F:
                                         dt * L + (ch + 1) * F],
                                func=AF.Exp,
                                scale=A_t[dt][:, n0 + j:n0 + j + 1])
                        bb = scanp.tile([128, BLK], FP16, tag="bb")
                        du_view = duu[dt][:, ch * F:(ch + 1) * F]
                        nc.vector.tensor_tensor(
                            out=bb[:].rearrange("p (g f) -> p g f", g=GN),
                            in0=du_view.unsqueeze(1).broadcast_to([128, GN, F]),
                            in1=b_bc[:].rearrange("p (g f) -> p g f", g=GN),
                            op=ALU.mult)
                        if chained:
                            tmp = colsp.tile([128, GN], FP16, tag="fix")
                            nc.vector.tensor_tensor(
                                out=tmp[:], in0=dA_g[:, :, 0],
                                in1=carry[2][dt][:, n0:n0 + GN], op=ALU.mult)
                            bst = bb[:].rearrange("p (g f) -> p g f", g=GN)[:, :, 0]
                            nc.vector.tensor_tensor(out=bst, in0=tmp[:],
                                                    in1=bst, op=ALU.add)
                        nc.vector.memset(dA_g[:, :, 0], 0.0)
                        hh = scanp.tile([128, BLK], FP16, tag="hh")
                        nc.vector.tensor_tensor_scan(
                            out=hh[:], data0=dA[:], data1=bb[:], initial=0.0,
                            op0=ALU.mult, op1=ALU.add)
                        if save_carry:
                            nc.vector.tensor_copy(
                                out=carry[2][dt][:, n0:n0 + GN],
                                in_=hh[:].rearrange("p (g f) -> p g f",
                                                    g=GN)[:, :, F - 1])
                        prod = scanp.tile([128, BLK], FP16, tag="prod")
                        nc.vector.tensor_tensor(
                            out=prod[:], in0=hh[:], in1=c_bc[:], op=ALU.mult)
                        for g in range(GN):
                            nc.tensor.matmul(
                                y_ps[dt][:], identT[:],
                                prod[:, g * F:(g + 1) * F],
                                start=(ng == 0 and g == 0),
                                stop=(ng == NG - 1 and g == GN - 1))

                # ---- post: z -> silu ; y3 = (y + D*u) * silu(z) ; out_proj
                c0 = tok_off + ch * F
                wiz = []
                for kt in range(4):
                    wt = wpool.tile([128, 512], FP16, tag="w_z")
                    nc.sync.dma_start(out=wt[:], in_=w_in[m][kt][:, 512:1024])
                    wiz.append(wt)
                y3s = []
                for dt in range(NDT):
                    psz = psmm.tile([128, 512], FP32, tag="ps")
                    for kt in range(4):
                        nc.tensor.matmul(psz[:], wiz[kt][:, dt * 128:(dt + 1) * 128],
                                         xnT[kt][:, c0:c0 + F],
                                         start=(kt == 0), stop=(kt == 3))
                    sgz = postp.tile([128, 512], FP16, tag="sgz", bufs=1)
                    nc.scalar.activation(out=sgz[:], in_=psz[:], func=AF.Sigmoid)
                    szt = postp.tile([128, 512], FP16, tag="sz")
                    nc.vector.tensor_tensor(out=szt[:], in0=psz[:], in1=sgz[:],
                                            op=ALU.mult)
                    y2 = postp.tile([128, 512], FP16, tag="y2")
                    nc.vector.scalar_tensor_tensor(
                        out=y2[:], in0=u_tiles[m][dt][:, ch * F:(ch + 1) * F],
                        scalar=D_t[(m, dt)][:], in1=y_ps[dt][:],
                        op0=ALU.mult, op1=ALU.add)
                    y3 = postp.tile([128, 512], FP16, tag="y3", bufs=4)
                    nc.vector.tensor_tensor(out=y3[:], in0=y2[:], in1=szt[:],
                                            op=ALU.mult)
                    y3s.append(y3)

                wo = []
                for kt in range(NDT):
                    wt = wpool.tile([128, 512], FP16, tag="w_out")
                    nc.sync.dma_start(out=wt[:], in_=w_out[m][kt])
                    wo.append(wt)
                so = []
                for ot in range(4):
                    ps = psmm.tile([128, 512], FP32, tag="ps")
                    for kt in range(NDT):
                        nc.tensor.matmul(ps[:], wo[kt][:, ot * 128:(ot + 1) * 128],
                                         y3s[kt][:], start=(kt == 0),
                                         stop=(kt == NDT - 1))
                    st = postp.tile([128, 512], FP16, tag="so", bufs=4)
                    nc.scalar.copy(out=st[:], in_=ps[:])
                    so.append(st)
                for tt in range(4):
                    ob = postp.tile([128, 512], FP16, tag="obuf")
                    for ot in range(4):
                        pst = psmm.tile([128, 128], FP16, tag="ps")
                        nc.tensor.transpose(pst[:],
                                            so[ot][:, tt * 128:(tt + 1) * 128],
                                            identT[:])
                        nc.scalar.copy(out=ob[:, ot * 128:(ot + 1) * 128],
                                       in_=pst[:])
                    nc.sync.dma_start(
                        out=rs_in[m][ch * F + tt * 128:ch * F + (tt + 1) * 128, :],
                        in_=ob[:])

            # ================= post-RS MLP (one 512-token chunk of branch m)
            def mlp_branch(m, ch=0):
                nc.gpsimd.collective_compute(
                    "ReduceScatter", ALU.add, replica_groups=GROUPS,
                    ins=[rs_in[m][ch * 512:(ch + 1) * 512, :]],
                    outs=[rs_out[(m, ch)].opt()])
                sec0, nrow = OUT_SECS[m]
                rsv = mlpp.tile([128, 512], FP16, tag="rsv")
                nc.sync.dma_start(out=rsv[:], in_=rs_out[(m, ch)][:])
                xqv = mlpp.tile([128, 512], FP32, tag="xqv")
                nc.sync.dma_start(
                    out=xqv[:],
                    in_=xq[sec0 + ch * 128:sec0 + (ch + 1) * 128, :])
                resid = mlpp.tile([128, 512], FP32, tag="resid")
                nc.vector.tensor_tensor(out=resid[:], in0=rsv[:], in1=xqv[:],
                                        op=ALU.add)
                mlp_block(resid, w_fc1[m], HID_SMALL, w_fc2[m],
                          sec0 + ch * 128)

            # ---------------- schedule (program order guides the Tile
            # scheduler: each branch's AllReduce flies while the previous
            # branch's scan keeps the Vector engine saturated)
            pre_branch(0)
            xq_text = mlpp.tile([128, 512], FP32, tag="xq_text")
            nc.sync.dma_start(out=xq_text[:], in_=xq[512:640, :])
            mlp_block(xq_text, w_fc1_t, HID_TEXT, w_fc2_t, 512)
            pre_branch(1)
            scan_chunk(0, 0)
            pre_branch(2)
            scan_chunk(1, 0)
            mlp_branch(0)
            scan_chunk(2, 0)
            mlp_branch(1)
            mlp_branch(2, 0)
            scan_chunk(2, 1)
            mlp_branch(2, 1)

    nc.finalize()
    return nc


_PROGRAM = None


def _get_program():
    global _PROGRAM
    if _PROGRAM is None:
        _PROGRAM = build_program()
    return _PROGRAM


def _np(a, dt=np.float32):
    return np.ascontiguousarray(np.asarray(a), dtype=dt)


def prepare_in_maps(x, params):
    x = _np(x)
    norms = {k: _np(v) for k, v in params["norms"].items()}
    mamba = [{k: _np(v) for k, v in params[mk].items()} for mk in MAMBA_KEYS]
    mlps = [{k: _np(v) for k, v in params[mk].items()} for mk in MLP_KEYS]
    textp = {k: _np(v) for k, v in params["text_mlp"].items()}
    ident = np.eye(128, dtype=np.float16)
    norm_fold = [
        (norms["music_1"], norms["music_2"]),
        (norms["up_1"], norms["up_2"]),
        (norms["down_1"], norms["down_2"]),
    ]
    A_neg = -np.exp(mamba[0]["A_log"])
    for mmx in mamba[1:]:
        assert np.allclose(-np.exp(mmx["A_log"]), A_neg), \
            "mamba A_log differ across branches; not supported"
    for mm in mamba:
        assert np.all(mm["conv_b"] == 0), "conv bias not folded"
    for gm in mlps + [textp]:
        assert np.all(gm["fc1_b"] == 0) and np.all(gm["fc2_b"] == 0)

    in_maps = []
    for g in range(2):
        xg = x[g]
        x_t16 = np.ascontiguousarray(
            xg.astype(np.float16).reshape(16, 128, 512))
        for r in range(R_SH):
            d0 = r * DI_SH
            im = {"x_t": x_t16, "ident": ident}
            im["xq"] = np.ascontiguousarray(np.concatenate([
                xg[r * 128:(r + 1) * 128],
                xg[512 + r * 128:512 + (r + 1) * 128],
                xg[1024 + r * 128:1024 + (r + 1) * 128],
                xg[1536 + r * 128:1536 + (r + 1) * 128],
                xg[1536 + r * 128:1536 + (r + 1) * 128],
            ], axis=0), dtype=np.float32)
            im["A_bc"] = np.ascontiguousarray(
                A_neg[d0:d0 + DI_SH].reshape(NDT, 128, 128), dtype=np.float32)
            im["dtb"] = np.ascontiguousarray(
                np.stack([mamba[m]["dt_proj_b"][d0:d0 + DI_SH]
                          .reshape(NDT, 128, 1) for m in range(3)]),
                dtype=np.float32)
            im["D_sh"] = np.ascontiguousarray(
                np.stack([mamba[m]["D"][d0:d0 + DI_SH]
                          .reshape(NDT, 128, 1) for m in range(3)]),
                dtype=np.float32)
            for m in range(3):
                mm = mamba[m]
                w1n, w2n = norm_fold[m]
                Win = mm["in_proj_w"] * w1n[None, :]
                Wsl = np.concatenate(
                    [Win[d0:d0 + DI_SH],
                     Win[D_INNER + d0:D_INNER + d0 + DI_SH]], axis=0)
                im[f"w_in_{m}"] = np.ascontiguousarray(
                    Wsl.T.reshape(4, 128, 1024), dtype=np.float16)
                im[f"w_conv_{m}"] = np.ascontiguousarray(
                    mm["conv_w"][d0:d0 + DI_SH].reshape(NDT, 128, D_CONV),
                    dtype=np.float32)
                im[f"w_xp_{m}"] = np.ascontiguousarray(
                    mm["x_proj_w"][:, d0:d0 + DI_SH].T.reshape(NDT, 128, 288),
                    dtype=np.float16)
                im[f"w_dt_{m}"] = np.ascontiguousarray(
                    mm["dt_proj_w"][d0:d0 + DI_SH].T, dtype=np.float16)
                im[f"w_out_{m}"] = np.ascontiguousarray(
                    mm["out_proj_w"][:, d0:d0 + DI_SH].T.reshape(NDT, 128, 512),
                    dtype=np.float16)
                gm = mlps[m]
                F1 = gm["fc1_w"] * w2n[None, :]
                im[f"w_fc1_{m}"] = np.ascontiguousarray(
                    F1.T.reshape(4, 128, 2 * HID_SMALL), dtype=np.float16)
                im[f"w_fc2_{m}"] = np.ascontiguousarray(
                    gm["fc2_w"].T.reshape(HID_SMALL // 128, 128, 512),
                    dtype=np.float16)
            tw = norms[TEXT_SEGS[r]]
            T1 = textp["fc1_w"] * tw[None, :]
            im["w_fc1_t"] = np.ascontiguousarray(
                T1.T.reshape(4, 128, 2 * HID_TEXT), dtype=np.float16)
            im["w_fc2_t"] = np.ascontiguousarray(
                textp["fc2_w"].T.reshape(HID_TEXT // 128, 128, 512),
                dtype=np.float16)
            in_maps.append(im)
    return in_maps


def assemble(results):
    out = np.zeros((2, OUT_TOTAL, D_MODEL), dtype=np.float32)
    for g in range(2):
        for r in range(R_SH):
            o = results[g * R_SH + r]["out"]
            out[g, r * 128:(r + 1) * 128] = o[0:128]
            out[g, 512 + r * 128:512 + (r + 1) * 128] = o[128:256]
            out[g, 1024 + r * 128:1024 + (r + 1) * 128] = o[256:384]
            out[g, 1536 + r * 128:1536 + (r + 1) * 128] = o[384:512]
            out[g, 2048 + r * 128:2048 + (r + 1) * 128] = o[512:640]
    return out


def kernel(x, T_motion, text_meta, params):
    assert int(T_motion) == T, f"kernel compiled for T_motion={T}"
    nc = _get_program()
    in_maps = prepare_in_maps(x, params)
    res = run_bass_kernel_spmd(nc, in_maps, core_ids=list(range(8)))
    return assemble(res.results)


# revision 27
# speedup vs baseline: 1.0122x; 1.0004x over previous
"""Trainium2 Bass kernel for nn_Block_Head_83047487635490 (Mamba motion block).

Sharding: 8 cores = 2 batch groups x 4-way tensor-parallel (d_inner 2048 -> 512
per core).  Per group: mamba in_proj/conv/x_proj/dt_proj/scan/out_proj sharded
over d_inner with a small fp16 all-reduce of x_db (288 x L) and a
reduce-scatter of out_proj partials over tokens; the gated MLPs and the text
MLP run token-parallel (full hidden) on the scattered quarters.

Selective scan: per (chunk, dtile, n-group) the recurrence
h = exp(delta*A_n) * h + (delta*u) * B_n runs as a single fp16
tensor_tensor_scan over Gn concatenated n-blocks (decay zeroed at block
starts resets the state); y = sum_n h*C_n via PE identity-matmul accumulation
in PSUM.
"""

import sys

if "/opt/trn_rl_repo" not in sys.path:
    sys.path.insert(0, "/opt/trn_rl_repo")

import numpy as np

from concourse import bacc, bass, mybir, tile
from concourse.bass_utils import run_bass_kernel_spmd

# ---------------------------------------------------------------- constants
B = 2
T = 512
N_TEXT = 128
TOTAL = 2048          # 3*T + 4*N_TEXT
D_MODEL = 512
D_STATE = 128
D_CONV = 4
D_INNER = 2048
DT_RANK = 32
HID_SMALL = 768
HID_TEXT = 2048
EPS = 1e-6
OUT_TOTAL = 2560      # music 512 + up 512 + down 1024 + text 512

R_SH = 4              # TP degree
DI_SH = D_INNER // R_SH
NDT = DI_SH // 128    # 4 d-tiles per core
F = 512               # time chunk
GN = 4                # n values per scan instruction
NG = D_STATE // GN    # 32
BLK = GN * F

BRANCHES = [(0, 512, 1), (512, 512, 1), (1024, 1024, 2)]  # (tok_off, L, chunks)
MLP_KEYS = ["gate_mlp_1", "gate_mlp_2", "gate_mlp_3"]
TEXT_SEGS = ["text_upper", "text_lower", "text_torso", "text_whole"]
MAMBA_KEYS = ["mamba_music", "mamba_up", "mamba_down"]

FP16 = mybir.dt.float16
FP32 = mybir.dt.float32
AF = mybir.ActivationFunctionType
ALU = mybir.AluOpType

OUT_SECS = [(0, 128), (128, 128), (256, 256), (512, 128)]
GROUPS = [[0, 1, 2, 3], [4, 5, 6, 7]]


def build_program():
    nc = bacc.Bacc("TRN2", target_bir_lowering=False, debug=False, num_devices=8)

    def din(name, shape, dt=FP16):
        return nc.dram_tensor(name, list(shape), dt, kind="ExternalInput").ap()

    x_t = din("x_t", (16, 128, 512))
    xq = din("xq", (640, 512), FP32)
    ident_in = din("ident", (128, 128))
    A_in = din("A_bc", (NDT, 128, 128), FP32)
    dtb_in = din("dtb", (3, NDT, 128, 1), FP32)
    D_in = din("D_sh", (3, NDT, 128, 1), FP32)
    w_in = [din(f"w_in_{m}", (4, 128, 1024)) for m in range(3)]
    w_conv = [din(f"w_conv_{m}", (NDT, 128, D_CONV), FP32) for m in range(3)]
    w_xp = [din(f"w_xp_{m}", (NDT, 128, 288)) for m in range(3)]
    w_dt = [din(f"w_dt_{m}", (DT_RANK, DI_SH)) for m in range(3)]
    w_out = [din(f"w_out_{m}", (NDT, 128, 512)) for m in range(3)]
    w_fc1 = [din(f"w_fc1_{m}", (4, 128, 2 * HID_SMALL)) for m in range(3)]
    w_fc2 = [din(f"w_fc2_{m}", (HID_SMALL // 128, 128, 512)) for m in range(3)]
    w_fc1_t = din("w_fc1_t", (4, 128, 2 * HID_TEXT))
    w_fc2_t = din("w_fc2_t", (HID_TEXT // 128, 128, 512))
    out_d = nc.dram_tensor("out", [640, 512], FP32, kind="ExternalOutput").ap()

    with tile.TileContext(nc) as tc:
        with (
            tc.tile_pool(name="const", bufs=1) as constp,
            tc.tile_pool(name="persist", bufs=1) as persist,
            tc.tile_pool(name="small", bufs=2) as small,
            tc.tile_pool(name="cols", bufs=4) as colsp,
            tc.tile_pool(name="wpool", bufs=4) as wpool,
            tc.tile_pool(name="wmlp", bufs=4) as wmlpp,
            tc.tile_pool(name="branch", bufs=1) as branchp,
            tc.tile_pool(name="scan", bufs=3) as scanp,
            tc.tile_pool(name="bc", bufs=4) as bcp,
            tc.tile_pool(name="post", bufs=2) as postp,
            tc.tile_pool(name="mlp", bufs=1) as mlpp,
            tc.tile_pool(name="hpool", bufs=16) as hpoolp,
            tc.tile_pool(name="psmm", bufs=4, space="PSUM") as psmm,
            tc.tile_pool(name="ypsum", bufs=1, space="PSUM") as ypsum,
            tc.tile_pool(name="dram", bufs=1, space="DRAM") as dramp,
        ):
            # ---------------- constants
            identT = constp.tile([128, 128], FP16, tag="ident")
            nc.sync.dma_start(out=identT[:], in_=ident_in[:])
            epscol = constp.tile([128, 1], FP32, tag="epscol")
            nc.vector.memset(epscol[:], EPS)
            A_t, dtb_t, D_t = [], {}, {}
            for dt in range(NDT):
                a = constp.tile([128, 128], FP32, tag=f"A{dt}")
                nc.sync.dma_start(out=a[:], in_=A_in[dt])
                A_t.append(a)
            for m in range(3):
                for dt in range(NDT):
                    bcol = constp.tile([128, 1], FP32, tag=f"dtb{m}_{dt}")
                    nc.sync.dma_start(out=bcol[:], in_=dtb_in[m, dt])
                    dtb_t[(m, dt)] = bcol
                    dcol = constp.tile([128, 1], FP32, tag=f"D{m}_{dt}")
                    nc.sync.dma_start(out=dcol[:], in_=D_in[m, dt])
                    D_t[(m, dt)] = dcol

            # ---------------- helpers
            def rms_tile(src_tile, dst_tile):
                scratch = small.tile([128, 512], FP16, tag="rms_scratch", bufs=1)
                ssq = colsp.tile([128, 1], FP32, tag="rms_ssq")
                nc.scalar.activation(out=scratch[:], in_=src_tile[:],
                                     func=AF.Square, accum_out=ssq[:])
                rr = colsp.tile([128, 1], FP32, tag="rms_rr")
                nc.scalar.activation(out=rr[:], in_=ssq[:], func=AF.Sqrt,
                                     scale=1.0 / D_MODEL, bias=epscol[:])
                inv = colsp.tile([128, 1], FP32, tag="rms_inv")
                nc.vector.reciprocal(out=inv[:], in_=rr[:])
                nc.vector.tensor_scalar(out=dst_tile[:], in0=src_tile[:],
                                        scalar1=inv[:], scalar2=None,
                                        op0=ALU.mult)

            def transpose_into(src_tile, n_blocks, put_block):
                for bb in range(n_blocks):
                    pst = psmm.tile([128, 128], FP16, tag="ps")
                    nc.tensor.transpose(pst[:], src_tile[:, bb * 128:(bb + 1) * 128],
                                        identT[:])
                    put_block(bb, pst)

            # ---------------- prolog: x -> rmsnorm -> transpose -> xnT
            xnT = [persist.tile([128, TOTAL], FP16, tag=f"xnT{k}",
                                name=f"xnT{k}") for k in range(4)]
            for i in range(16):
                xt = postp.tile([128, 512], FP16, tag="so", bufs=4, name="xt")
                nc.sync.dma_start(out=xt[:], in_=x_t[i])
                xn = postp.tile([128, 512], FP16, tag="y2", name="xn")
                rms_tile(xt, xn)

                def put(kt, pst, i=i):
                    nc.scalar.copy(out=xnT[kt][:, i * 128:(i + 1) * 128],
                                   in_=pst[:])
                transpose_into(xn, 4, put)

            # ---------------- generic gated-MLP block on one (128, 512) resid
            def mlp_block(resid_tile, w1_ap, hid, w2_ap, out_row):
                xn2 = mlpp.tile([128, 512], FP16, tag="mlp_xn")
                rms_tile(resid_tile, xn2)
                xn2T = []
                for kt in range(4):
                    dstt = mlpp.tile([128, 128], FP16, tag=f"mlp_xnT{kt}")
                    xn2T.append(dstt)

                def putx(kt, pst):
                    nc.scalar.copy(out=xn2T[kt][:], in_=pst[:])
                transpose_into(xn2, 4, putx)

                n_h = hid // 128
                hh = []
                for oy in range(n_h):
                    ps_y = psmm.tile([128, 128], FP32, tag="ps")
                    ps_g = psmm.tile([128, 128], FP32, tag="ps")
                    for half, ps in ((0, ps_y), (1, ps_g)):
                        o = oy + half * n_h
                        for kt in range(4):
                            w = wmlpp.tile([128, 128], FP16, tag="wm1")
                            nc.sync.dma_start(
                                out=w[:], in_=w1_ap[kt][:, o * 128:(o + 1) * 128])
                            nc.tensor.matmul(ps[:], w[:], xn2T[kt][:],
                                             start=(kt == 0), stop=(kt == 3))
                    sg = mlpp.tile([128, 128], FP16, tag="mlp_sg")
                    nc.scalar.activation(out=sg[:], in_=ps_g[:], func=AF.Sigmoid)
                    sg2 = mlpp.tile([128, 128], FP16, tag="mlp_sg2")
                    nc.vector.tensor_tensor(out=sg2[:], in0=ps_g[:], in1=sg[:],
                                            op=ALU.mult)
                    ht = hpoolp.tile([128, 128], FP16, tag="mlp_h")
                    nc.vector.tensor_tensor(out=ht[:], in0=ps_y[:], in1=sg2[:],
                                            op=ALU.mult)
                    hh.append(ht)
                mT = mlpp.tile([128, 512], FP16, tag="mlp_mT")
                for ot in range(4):
                    ps = psmm.tile([128, 128], FP32, tag="ps")
                    for kt in range(n_h):
                        w = wmlpp.tile([128, 128], FP16, tag="wm2")
                        nc.sync.dma_start(
                            out=w[:], in_=w2_ap[kt][:, ot * 128:(ot + 1) * 128])
                        nc.tensor.matmul(ps[:], w[:], hh[kt][:],
                                         start=(kt == 0), stop=(kt == n_h - 1))
                    mo = mlpp.tile([128, 128], FP16, tag="mlp_mo")
                    nc.scalar.copy(out=mo[:], in_=ps[:])
                    pst = psmm.tile([128, 128], FP16, tag="ps")
                    nc.tensor.transpose(pst[:], mo[:], identT[:])
                    nc.scalar.copy(out=mT[:, ot * 128:(ot + 1) * 128], in_=pst[:])
                fin = mlpp.tile([128, 512], FP32, tag="mlp_fin")
                nc.vector.tensor_tensor(out=fin[:], in0=mT[:], in1=resid_tile[:],
                                        op=ALU.add)
                nc.sync.dma_start(out=out_d[out_row:out_row + 128, :], in_=fin[:])

            # ---------------- per-branch state
            ar_out = {}
            u_tiles = {}
            du_tiles = {}
            delta_all = {}
            carry = {}
            rs_in = {}
            rs_out = {}
            for m in range(3):
                L = BRANCHES[m][1]
                rs_in[m] = dramp.tile([L, 512], FP16, tag=f"rsin{m}",
                                      name=f"rsin{m}")
                for ch in range(L // 512):
                    rs_out[(m, ch)] = dramp.tile(
                        [128, 512], FP16, tag=f"rsout{m}_{ch}",
                        name=f"rsout{m}_{ch}")

            # ================= pre-scan stage
            def pre_branch(m):
                tok_off, L, n_ch = BRANCHES[m]
                wi = []
                for kt in range(4):
                    wt = wpool.tile([128, 512], FP16, tag="w_in")
                    nc.sync.dma_start(out=wt[:], in_=w_in[m][kt][:, 0:512])
                    wi.append(wt)
                wxp = []
                for kt in range(NDT):
                    wt = wpool.tile([128, 288], FP16, tag="w_xp")
                    nc.sync.dma_start(out=wt[:], in_=w_xp[m][kt])
                    wxp.append(wt)
                wdt = wpool.tile([DT_RANK, DI_SH], FP16, tag="w_dt")
                nc.sync.dma_start(out=wdt[:], in_=w_dt[m][:])
                wcv = []
                for dt in range(NDT):
                    wt = wpool.tile([128, D_CONV], FP32, tag="w_conv")
                    nc.sync.dma_start(out=wt[:], in_=w_conv[m][dt])
                    wcv.append(wt)

                xiT = [branchp.tile([128, L + 3], FP16, tag=f"xiT{dt}_{m}",
                                    name=f"xiT{dt}_{m}") for dt in range(NDT)]
                uu = [xiT[dt][:, 3:3 + L] for dt in range(NDT)]
                duu = [branchp.tile([128, L], FP16, tag=f"du{dt}_{m}",
                                    name=f"du{dt}_{m}") for dt in range(NDT)]
                dall = branchp.tile([128, NDT * L], FP16, tag=f"delta_{m}",
                                    name=f"delta_{m}")

                for dt in range(NDT):
                    nc.vector.memset(xiT[dt][:, 0:3], 0.0)

                # in_proj (xi half only; z recomputed at post time)
                for ch in range(n_ch):
                    c0 = tok_off + ch * F
                    for ot in range(4):
                        ps = psmm.tile([128, 512], FP32, tag="ps")
                        for kt in range(4):
                            nc.tensor.matmul(ps[:], wi[kt][:, ot * 128:(ot + 1) * 128],
                                             xnT[kt][:, c0:c0 + F],
                                             start=(kt == 0), stop=(kt == 3))
                        nc.scalar.copy(
                            out=xiT[ot][:, 3 + ch * F:3 + (ch + 1) * F], in_=ps[:])

                # conv + silu -> u
                for dt in range(NDT):
                    acc0 = small.tile([128, 1024], FP16, tag="conv_a", bufs=1)
                    acc1 = small.tile([128, 1024], FP16, tag="conv_b", bufs=1)
                    nc.vector.tensor_scalar(out=acc0[:, 0:L], in0=xiT[dt][:, 0:L],
                                            scalar1=wcv[dt][:, 0:1], scalar2=None,
                                            op0=ALU.mult)
                    a, b_ = acc0, acc1
                    for j in range(1, D_CONV):
                        nc.vector.scalar_tensor_tensor(
                            out=b_[:, 0:L], in0=xiT[dt][:, j:j + L],
                            scalar=wcv[dt][:, j:j + 1], in1=a[:, 0:L],
                            op0=ALU.mult, op1=ALU.add)
                        a, b_ = b_, a
                    nc.scalar.activation(out=b_[:, 0:L], in_=a[:, 0:L],
                                         func=AF.Sigmoid)
                    nc.vector.tensor_tensor(out=uu[dt][:, 0:L], in0=a[:, 0:L],
                                            in1=b_[:, 0:L], op=ALU.mult)

                # x_proj partials -> DRAM -> AllReduce (fp16)
                arin = dramp.tile([288, L], FP16, tag=f"arin{m}")
                arout = dramp.tile([288, L], FP16, tag=f"arout{m}")
                for ch in range(n_ch):
                    for po, pw in ((0, 128), (128, 128), (256, 32)):
                        ps = psmm.tile([128, 512], FP32, tag="ps")
                        for kt in range(NDT):
                            nc.tensor.matmul(
                                ps[:pw, :], wxp[kt][:, po:po + pw],
                                uu[kt][:, ch * F:(ch + 1) * F],
                                start=(kt == 0), stop=(kt == NDT - 1))
                        sb = small.tile([128, 512], FP16, tag="xdb_sb")
                        nc.scalar.copy(out=sb[:pw, :], in_=ps[:pw, :])
                        nc.sync.dma_start(
                            out=arin[po:po + pw, ch * F:(ch + 1) * F],
                            in_=sb[:pw, :])
                nc.gpsimd.collective_compute(
                    "AllReduce", ALU.add, replica_groups=GROUPS,
                    ins=[arin.opt()], outs=[arout.opt()])

                # dt_proj + softplus -> delta ; du = delta * u
                dtT = small.tile([DT_RANK, 1024], FP16, tag="dtT", bufs=1)
                nc.sync.dma_start(out=dtT[:, 0:L], in_=arout[0:DT_RANK, :])
                for dt in range(NDT):
                    for ch in range(n_ch):
                        ps = psmm.tile([128, 512], FP32, tag="ps")
                        nc.tensor.matmul(ps[:], wdt[:, dt * 128:(dt + 1) * 128],
                                         dtT[:, ch * F:(ch + 1) * F],
                                         start=True, stop=True)
                        spe = small.tile([128, 512], FP16, tag="spe", bufs=1)
                        nc.scalar.activation(out=spe[:], in_=ps[:], func=AF.Exp,
                                             bias=dtb_t[(m, dt)][:])
                        nc.scalar.activation(
                            out=dall[:, dt * L + ch * F: dt * L + (ch + 1) * F],
                            in_=spe[:], func=AF.Ln, bias=1.0)
                    nc.vector.tensor_tensor(out=duu[dt][:, 0:L],
                                            in0=dall[:, dt * L:dt * L + L],
                                            in1=uu[dt][:, 0:L], op=ALU.mult)

                ar_out[m] = arout
                u_tiles[m] = uu
                du_tiles[m] = duu
                delta_all[m] = dall
                if m == 2:
                    carry[m] = [branchp.tile([128, 128], FP16, tag=f"carry{dt}",
                                             name=f"carry{dt}")
                                for dt in range(NDT)]

            # ================= scan + post of one chunk
            def scan_chunk(m, ch):
                tok_off, L, n_ch = BRANCHES[m]
                arout = ar_out[m]
                dall = delta_all[m]
                duu = du_tiles[m]
                chained = (m == 2 and ch == 1)
                save_carry = (m == 2 and ch == 0)

                y_ps = [ypsum.tile([128, 512], FP32, tag=f"yps{dt}",
                                   name=f"yps{dt}_{m}_{ch}")
                        for dt in range(NDT)]
                for ng in range(NG):
                    n0 = ng * GN
                    b_bc = bcp.tile([128, BLK], FP16, tag="b_bc")
                    src = arout[DT_RANK + n0:DT_RANK + n0 + GN,
                                ch * F:(ch + 1) * F]
                    nc.sync.dma_start(
                        out=b_bc[:].rearrange("p (g f) -> p g f", g=GN),
                        in_=src.unsqueeze(0).broadcast_to([128, GN, F]))
                    c_bc = bcp.tile([128, BLK], FP16, tag="c_bc")
                    src = arout[DT_RANK + D_STATE + n0:
                                DT_RANK + D_STATE + n0 + GN,
                                ch * F:(ch + 1) * F]
                    nc.sync.dma_start(
                        out=c_bc[:].rearrange("p (g f) -> p g f", g=GN),
                        in_=src.unsqueeze(0).broadcast_to([128, GN, F]))

                    for dt in range(NDT):
                        dA = scanp.tile([128, BLK], FP16, tag="dA", bufs=2)
                        dA_g = dA[:].rearrange("p (g f) -> p g f", g=GN)
                        for j in range(GN):
                            nc.scalar.activation(
                                out=dA_g[:, j, :],
                                in_=dall[:, dt * L + ch * F:
                                         dt * L + (ch + 1) * F],
                                func=AF.Exp,
                                scale=A_t[dt][:, n0 + j:n0 + j + 1])
                        bb = scanp.tile([128, BLK], FP16, tag="bb")
                        du_view = duu[dt][:, ch * F:(ch + 1) * F]
                        nc.vector.tensor_tensor(
                            out=bb[:].rearrange("p (g f) -> p g f", g=GN),
                            in0=du_view.unsqueeze(1).broadcast_to([128, GN, F]),
                            in1=b_bc[:].rearrange("p (g f) -> p g f", g=GN),
                            op=ALU.mult)
                        if chained:
                            tmp = colsp.tile([128, GN], FP16, tag="fix")
                            nc.vector.tensor_tensor(
                                out=tmp[:], in0=dA_g[:, :, 0],
                                in1=carry[2][dt][:, n0:n0 + GN], op=ALU.mult)
                            bst = bb[:].rearrange("p (g f) -> p g f", g=GN)[:, :, 0]
                            nc.vector.tensor_tensor(out=bst, in0=tmp[:],
                                                    in1=bst, op=ALU.add)
                        nc.scalar.mul(out=dA_g[:, :, 0], in_=dA_g[:, :, 0],
                                      mul=0.0)
                        hh = scanp.tile([128, BLK], FP16, tag="hh")
                        nc.vector.tensor_tensor_scan(
                            out=hh[:], data0=dA[:], data1=bb[:], initial=0.0,
                            op0=ALU.mult, op1=ALU.add)
                        if save_carry:
                            nc.vector.tensor_copy(
                                out=carry[2][dt][:, n0:n0 + GN],
                                in_=hh[:].rearrange("p (g f) -> p g f",
                                                    g=GN)[:, :, F - 1])
                        prod = scanp.tile([128, BLK], FP16, tag="prod")
                        nc.vector.tensor_tensor(
                            out=prod[:], in0=hh[:], in1=c_bc[:], op=ALU.mult)
                        for g in range(GN):
                            nc.tensor.matmul(
                                y_ps[dt][:], identT[:],
                                prod[:, g * F:(g + 1) * F],
                                start=(ng == 0 and g == 0),
                                stop=(ng == NG - 1 and g == GN - 1))

                # ---- post: z -> silu ; y3 = (y + D*u) * silu(z) ; out_proj
                c0 = tok_off + ch * F
                wiz = []
                for kt in range(4):
                    wt = wpool.tile([128, 512], FP16, tag="w_z")
                    nc.sync.dma_start(out=wt[:], in_=w_in[m][kt][:, 512:1024])
                    wiz.append(wt)
                y3s = []
                for dt in range(NDT):
                    psz = psmm.tile([128, 512], FP32, tag="ps")
                    for kt in range(4):
                        nc.tensor.matmul(psz[:], wiz[kt][:, dt * 128:(dt + 1) * 128],
                                         xnT[kt][:, c0:c0 + F],
                                         start=(kt == 0), stop=(kt == 3))
                    sgz = postp.tile([128, 512], FP16, tag="sgz", bufs=1)
                    nc.scalar.activation(out=sgz[:], in_=psz[:], func=AF.Sigmoid)
                    szt = postp.tile([128, 512], FP16, tag="sz")
                    nc.vector.tensor_tensor(out=szt[:], in0=psz[:], in1=sgz[:],
                                            op=ALU.mult)
                    y2 = postp.tile([128, 512], FP16, tag="y2")
                    nc.vector.scalar_tensor_tensor(
                        out=y2[:], in0=u_tiles[m][dt][:, ch * F:(ch + 1) * F],
                        scalar=D_t[(m, dt)][:], in1=y_ps[dt][:],
                        op0=ALU.mult, op1=ALU.add)
                    y3 = postp.tile([128, 512], FP16, tag="y3", bufs=4)
                    nc.vector.tensor_tensor(out=y3[:], in0=y2[:], in1=szt[:],
                                            op=ALU.mult)
                    y3s.append(y3)

                wo = []
                for kt in range(NDT):
                    wt = wpool.tile([128, 512], FP16, tag="w_out")
                    nc.sync.dma_start(out=wt[:], in_=w_out[m][kt])
                    wo.append(wt)
                so = []
                for ot in range(4):
                    ps = psmm.tile([128, 512], FP32, tag="ps")
                    for kt in range(NDT):
                        nc.tensor.matmul(ps[:], wo[kt][:, ot * 128:(ot + 1) * 128],
                                         y3s[kt][:], start=(kt == 0),
                                         stop=(kt == NDT - 1))
                    st = postp.tile([128, 512], FP16, tag="so", bufs=4)
                    nc.scalar.copy(out=st[:], in_=ps[:])
                    so.append(st)
                for tt in range(4):
                    ob = postp.tile([128, 512], FP16, tag="obuf")
                    for ot in range(4):
                        pst = psmm.tile([128, 128], FP16, tag="ps")
                        nc.tensor.transpose(pst[:],
                                            so[ot][:, tt * 128:(tt + 1) * 128],
                                            identT[:])
                        nc.scalar.copy(out=ob[:, ot * 128:(ot + 1) * 128],
                                       in_=pst[:])
                    nc.sync.dma_start(
                        out=rs_in[m][ch * F + tt * 128:ch * F + (tt + 1) * 128, :],
                        in_=ob[:])

            # ================= post-RS MLP (one 512-token chunk of branch m)
            def mlp_branch(m, ch=0):
                nc.gpsimd.collective_compute(
                    "ReduceScatter", ALU.add, replica_groups=GROUPS,
                    ins=[rs_in[m][ch * 512:(ch + 1) * 512, :]],
                    outs=[rs_out[(m, ch)].opt()])
                sec0, nrow = OUT_SECS[m]
                rsv = mlpp.tile([128, 512], FP16, tag="rsv")
                nc.sync.dma_start(out=rsv[:], in_=rs_out[(m, ch)][:])
                xqv = mlpp.tile([128, 512], FP32, tag="xqv")
                nc.sync.dma_start(
                    out=xqv[:],
                    in_=xq[sec0 + ch * 128:sec0 + (ch + 1) * 128, :])
                resid = mlpp.tile([128, 512], FP32, tag="resid")
                nc.vector.tensor_tensor(out=resid[:], in0=rsv[:], in1=xqv[:],
                                        op=ALU.add)
                mlp_block(resid, w_fc1[m], HID_SMALL, w_fc2[m],
                          sec0 + ch * 128)

            # ---------------- schedule (program order guides the Tile
            # scheduler: each branch's AllReduce flies while the previous
            # branch's scan keeps the Vector engine saturated)
            pre_branch(0)
            xq_text = mlpp.tile([128, 512], FP32, tag="xq_text")
            nc.sync.dma_start(out=xq_text[:], in_=xq[512:640, :])
            mlp_block(xq_text, w_fc1_t, HID_TEXT, w_fc2_t, 512)
            pre_branch(1)
            scan_chunk(0, 0)
            pre_branch(2)
            scan_chunk(1, 0)
            mlp_branch(0)
            scan_chunk(2, 0)
            mlp_branch(1)
            mlp_branch(2, 0)
            scan_chunk(2, 1)
            mlp_branch(2, 1)

    nc.finalize()
    return nc


_PROGRAM = None


def _get_program():
    global _PROGRAM
    if _PROGRAM is None:
        _PROGRAM = build_program()
    return _PROGRAM


def _np(a, dt=np.float32):
    return np.ascontiguousarray(np.asarray(a), dtype=dt)


def prepare_in_maps(x, params):
    x = _np(x)
    norms = {k: _np(v) for k, v in params["norms"].items()}
    mamba = [{k: _np(v) for k, v in params[mk].items()} for mk in MAMBA_KEYS]
    mlps = [{k: _np(v) for k, v in params[mk].items()} for mk in MLP_KEYS]
    textp = {k: _np(v) for k, v in params["text_mlp"].items()}
    ident = np.eye(128, dtype=np.float16)
    norm_fold = [
        (norms["music_1"], norms["music_2"]),
        (norms["up_1"], norms["up_2"]),
        (norms["down_1"], norms["down_2"]),
    ]
    A_neg = -np.exp(mamba[0]["A_log"])
    for mmx in mamba[1:]:
        assert np.allclose(-np.exp(mmx["A_log"]), A_neg), \
            "mamba A_log differ across branches; not supported"
    for mm in mamba:
        assert np.all(mm["conv_b"] == 0), "conv bias not folded"
    for gm in mlps + [textp]:
        assert np.all(gm["fc1_b"] == 0) and np.all(gm["fc2_b"] == 0)

    in_maps = []
    for g in range(2):
        xg = x[g]
        x_t16 = np.ascontiguousarray(
            xg.astype(np.float16).reshape(16, 128, 512))
        for r in range(R_SH):
            d0 = r * DI_SH
            im = {"x_t": x_t16, "ident": ident}
            im["xq"] = np.ascontiguousarray(np.concatenate([
                xg[r * 128:(r + 1) * 128],
                xg[512 + r * 128:512 + (r + 1) * 128],
                xg[1024 + r * 128:1024 + (r + 1) * 128],
                xg[1536 + r * 128:1536 + (r + 1) * 128],
                xg[1536 + r * 128:1536 + (r + 1) * 128],
            ], axis=0), dtype=np.float32)
            im["A_bc"] = np.ascontiguousarray(
                A_neg[d0:d0 + DI_SH].reshape(NDT, 128, 128), dtype=np.float32)
            im["dtb"] = np.ascontiguousarray(
                np.stack([mamba[m]["dt_proj_b"][d0:d0 + DI_SH]
                          .reshape(NDT, 128, 1) for m in range(3)]),
                dtype=np.float32)
            im["D_sh"] = np.ascontiguousarray(
                np.stack([mamba[m]["D"][d0:d0 + DI_SH]
                          .reshape(NDT, 128, 1) for m in range(3)]),
                dtype=np.float32)
            for m in range(3):
                mm = mamba[m]
                w1n, w2n = norm_fold[m]
                Win = mm["in_proj_w"] * w1n[None, :]
                Wsl = np.concatenate(
                    [Win[d0:d0 + DI_SH],
                     Win[D_INNER + d0:D_INNER + d0 + DI_SH]], axis=0)
                im[f"w_in_{m}"] = np.ascontiguousarray(
                    Wsl.T.reshape(4, 128, 1024), dtype=np.float16)
                im[f"w_conv_{m}"] = np.ascontiguousarray(
                    mm["conv_w"][d0:d0 + DI_SH].reshape(NDT, 128, D_CONV),
                    dtype=np.float32)
                im[f"w_xp_{m}"] = np.ascontiguousarray(
                    mm["x_proj_w"][:, d0:d0 + DI_SH].T.reshape(NDT, 128, 288),
                    dtype=np.float16)
                im[f"w_dt_{m}"] = np.ascontiguousarray(
                    mm["dt_proj_w"][d0:d0 + DI_SH].T, dtype=np.float16)
                im[f"w_out_{m}"] = np.ascontiguousarray(
                    mm["out_proj_w"][:, d0:d0 + DI_SH].T.reshape(NDT, 128, 512),
                    dtype=np.float16)
                gm = mlps[m]
                F1 = gm["fc1_w"] * w2n[None, :]
                im[f"w_fc1_{m}"] = np.ascontiguousarray(
                    F1.T.reshape(4, 128, 2 * HID_SMALL), dtype=np.float16)
                im[f"w_fc2_{m}"] = np.ascontiguousarray(
                    gm["fc2_w"].T.reshape(HID_SMALL // 128, 128, 512),
                    dtype=np.float16)
            tw = norms[TEXT_SEGS[r]]
            T1 = textp["fc1_w"] * tw[None, :]
            im["w_fc1_t"] = np.ascontiguousarray(
                T1.T.reshape(4, 128, 2 * HID_TEXT), dtype=np.float16)
            im["w_fc2_t"] = np.ascontiguousarray(
                textp["fc2_w"].T.reshape(HID_TEXT // 128, 128, 512),
                dtype=np.float16)
            in_maps.append(im)
    return in_maps


def assemble(results):
    out = np.zeros((2, OUT_TOTAL, D_MODEL), dtype=np.float32)
    for g in range(2):
        for r in range(R_SH):
            o = results[g * R_SH + r]["out"]
            out[g, r * 128:(r + 1) * 128] = o[0:128]
            out[g, 512 + r * 128:512 + (r + 1) * 128] = o[128:256]
            out[g, 1024 + r * 128:1024 + (r + 1) * 128] = o[256:384]
            out[g, 1536 + r * 128:1536 + (r + 1) * 128] = o[384:512]
            out[g, 2048 + r * 128:2048 + (r + 1) * 128] = o[512:640]
    return out


def kernel(x, T_motion, text_meta, params):
    assert int(T_motion) == T, f"kernel compiled for T_motion={T}"
    nc = _get_program()
    in_maps = prepare_in_maps(x, params)
    res = run_bass_kernel_spmd(nc, in_maps, core_ids=list(range(8)))
    return assemble(res.results)


# revision 28
# speedup vs baseline: 1.0143x; 1.0021x over previous
"""Trainium2 Bass kernel for nn_Block_Head_83047487635490 (Mamba motion block).

Sharding: 8 cores = 2 batch groups x 4-way tensor-parallel (d_inner 2048 -> 512
per core).  Per group: mamba in_proj/conv/x_proj/dt_proj/scan/out_proj sharded
over d_inner with a small fp16 all-reduce of x_db (288 x L) and a
reduce-scatter of out_proj partials over tokens; the gated MLPs and the text
MLP run token-parallel (full hidden) on the scattered quarters.

Selective scan: per (chunk, dtile, n-group) the recurrence
h = exp(delta*A_n) * h + (delta*u) * B_n runs as a single fp16
tensor_tensor_scan over Gn concatenated n-blocks (decay zeroed at block
starts resets the state); y = sum_n h*C_n via PE identity-matmul accumulation
in PSUM.
"""

import sys

if "/opt/trn_rl_repo" not in sys.path:
    sys.path.insert(0, "/opt/trn_rl_repo")

import numpy as np

from concourse import bacc, bass, mybir, tile
from concourse.bass_utils import run_bass_kernel_spmd

# ---------------------------------------------------------------- constants
B = 2
T = 512
N_TEXT = 128
TOTAL = 2048          # 3*T + 4*N_TEXT
D_MODEL = 512
D_STATE = 128
D_CONV = 4
D_INNER = 2048
DT_RANK = 32
HID_SMALL = 768
HID_TEXT = 2048
EPS = 1e-6
OUT_TOTAL = 2560      # music 512 + up 512 + down 1024 + text 512

R_SH = 4              # TP degree
DI_SH = D_INNER // R_SH
NDT = DI_SH // 128    # 4 d-tiles per core
F = 512               # time chunk
GN = 4                # n values per scan instruction
NG = D_STATE // GN    # 32
BLK = GN * F

BRANCHES = [(0, 512, 1), (512, 512, 1), (1024, 1024, 2)]  # (tok_off, L, chunks)
MLP_KEYS = ["gate_mlp_1", "gate_mlp_2", "gate_mlp_3"]
TEXT_SEGS = ["text_upper", "text_lower", "text_torso", "text_whole"]
MAMBA_KEYS = ["mamba_music", "mamba_up", "mamba_down"]

FP16 = mybir.dt.float16
FP32 = mybir.dt.float32
AF = mybir.ActivationFunctionType
ALU = mybir.AluOpType

OUT_SECS = [(0, 128), (128, 128), (256, 256), (512, 128)]
GROUPS = [[0, 1, 2, 3], [4, 5, 6, 7]]


def build_program():
    nc = bacc.Bacc("TRN2", target_bir_lowering=False, debug=False, num_devices=8)

    def din(name, shape, dt=FP16):
        return nc.dram_tensor(name, list(shape), dt, kind="ExternalInput").ap()

    x_t = din("x_t", (16, 128, 512))
    xq = din("xq", (640, 512), FP32)
    ident_in = din("ident", (128, 128))
    A_in = din("A_bc", (NDT, 128, 128), FP32)
    dtb_in = din("dtb", (3, NDT, 128, 1), FP32)
    D_in = din("D_sh", (3, NDT, 128, 1), FP32)
    w_in = [din(f"w_in_{m}", (4, 128, 1024)) for m in range(3)]
    w_conv = [din(f"w_conv_{m}", (NDT, 128, D_CONV), FP32) for m in range(3)]
    w_xp = [din(f"w_xp_{m}", (NDT, 128, 288)) for m in range(3)]
    w_dt = [din(f"w_dt_{m}", (DT_RANK, DI_SH)) for m in range(3)]
    w_out = [din(f"w_out_{m}", (NDT, 128, 512)) for m in range(3)]
    w_fc1 = [din(f"w_fc1_{m}", (4, 128, 2 * HID_SMALL)) for m in range(3)]
    w_fc2 = [din(f"w_fc2_{m}", (HID_SMALL // 128, 128, 512)) for m in range(3)]
    w_fc1_t = din("w_fc1_t", (4, 128, 2 * HID_TEXT))
    w_fc2_t = din("w_fc2_t", (HID_TEXT // 128, 128, 512))
    out_d = nc.dram_tensor("out", [640, 512], FP32, kind="ExternalOutput").ap()

    with tile.TileContext(nc) as tc:
        with (
            tc.tile_pool(name="const", bufs=1) as constp,
            tc.tile_pool(name="persist", bufs=1) as persist,
            tc.tile_pool(name="small", bufs=2) as small,
            tc.tile_pool(name="cols", bufs=4) as colsp,
            tc.tile_pool(name="wpool", bufs=4) as wpool,
            tc.tile_pool(name="wmlp", bufs=4) as wmlpp,
            tc.tile_pool(name="branch", bufs=1) as branchp,
            tc.tile_pool(name="scan", bufs=3) as scanp,
            tc.tile_pool(name="bc", bufs=4) as bcp,
            tc.tile_pool(name="post", bufs=2) as postp,
            tc.tile_pool(name="mlp", bufs=1) as mlpp,
            tc.tile_pool(name="hpool", bufs=16) as hpoolp,
            tc.tile_pool(name="psmm", bufs=4, space="PSUM") as psmm,
            tc.tile_pool(name="ypsum", bufs=1, space="PSUM") as ypsum,
            tc.tile_pool(name="dram", bufs=1, space="DRAM") as dramp,
        ):
            # ---------------- constants
            identT = constp.tile([128, 128], FP16, tag="ident")
            nc.sync.dma_start(out=identT[:], in_=ident_in[:])
            epscol = constp.tile([128, 1], FP32, tag="epscol")
            nc.vector.memset(epscol[:], EPS)
            A_t, dtb_t, D_t = [], {}, {}
            for dt in range(NDT):
                a = constp.tile([128, 128], FP32, tag=f"A{dt}")
                nc.sync.dma_start(out=a[:], in_=A_in[dt])
                A_t.append(a)
            for m in range(3):
                for dt in range(NDT):
                    bcol = constp.tile([128, 1], FP32, tag=f"dtb{m}_{dt}")
                    nc.sync.dma_start(out=bcol[:], in_=dtb_in[m, dt])
                    dtb_t[(m, dt)] = bcol
                    dcol = constp.tile([128, 1], FP32, tag=f"D{m}_{dt}")
                    nc.sync.dma_start(out=dcol[:], in_=D_in[m, dt])
                    D_t[(m, dt)] = dcol

            # ---------------- helpers
            def rms_tile(src_tile, dst_tile):
                scratch = small.tile([128, 512], FP16, tag="rms_scratch", bufs=1)
                ssq = colsp.tile([128, 1], FP32, tag="rms_ssq")
                nc.scalar.activation(out=scratch[:], in_=src_tile[:],
                                     func=AF.Square, accum_out=ssq[:])
                rr = colsp.tile([128, 1], FP32, tag="rms_rr")
                nc.scalar.activation(out=rr[:], in_=ssq[:], func=AF.Sqrt,
                                     scale=1.0 / D_MODEL, bias=epscol[:])
                inv = colsp.tile([128, 1], FP32, tag="rms_inv")
                nc.vector.reciprocal(out=inv[:], in_=rr[:])
                nc.vector.tensor_scalar(out=dst_tile[:], in0=src_tile[:],
                                        scalar1=inv[:], scalar2=None,
                                        op0=ALU.mult)

            def transpose_into(src_tile, n_blocks, put_block):
                for bb in range(n_blocks):
                    pst = psmm.tile([128, 128], FP16, tag="ps")
                    nc.tensor.transpose(pst[:], src_tile[:, bb * 128:(bb + 1) * 128],
                                        identT[:])
                    put_block(bb, pst)

            # ---------------- prolog: x -> rmsnorm -> transpose -> xnT
            xnT = [persist.tile([128, TOTAL], FP16, tag=f"xnT{k}",
                                name=f"xnT{k}") for k in range(4)]
            for i in range(16):
                xt = postp.tile([128, 512], FP16, tag="so", bufs=4, name="xt")
                nc.sync.dma_start(out=xt[:], in_=x_t[i])
                xn = postp.tile([128, 512], FP16, tag="y2", name="xn")
                rms_tile(xt, xn)

                def put(kt, pst, i=i):
                    nc.scalar.copy(out=xnT[kt][:, i * 128:(i + 1) * 128],
                                   in_=pst[:])
                transpose_into(xn, 4, put)

            # ---------------- generic gated-MLP block on one (128, 512) resid
            def mlp_block(resid_tile, w1_ap, hid, w2_ap, out_row):
                xn2 = mlpp.tile([128, 512], FP16, tag="mlp_xn")
                rms_tile(resid_tile, xn2)
                xn2T = []
                for kt in range(4):
                    dstt = mlpp.tile([128, 128], FP16, tag=f"mlp_xnT{kt}")
                    xn2T.append(dstt)

                def putx(kt, pst):
                    nc.scalar.copy(out=xn2T[kt][:], in_=pst[:])
                transpose_into(xn2, 4, putx)

                n_h = hid // 128
                hh = []
                for oy in range(n_h):
                    ps_y = psmm.tile([128, 128], FP32, tag="ps")
                    ps_g = psmm.tile([128, 128], FP32, tag="ps")
                    for half, ps in ((0, ps_y), (1, ps_g)):
                        o = oy + half * n_h
                        for kt in range(4):
                            w = wmlpp.tile([128, 128], FP16, tag="wm1")
                            nc.sync.dma_start(
                                out=w[:], in_=w1_ap[kt][:, o * 128:(o + 1) * 128])
                            nc.tensor.matmul(ps[:], w[:], xn2T[kt][:],
                                             start=(kt == 0), stop=(kt == 3))
                    sg = mlpp.tile([128, 128], FP16, tag="mlp_sg")
                    nc.scalar.activation(out=sg[:], in_=ps_g[:], func=AF.Sigmoid)
                    sg2 = mlpp.tile([128, 128], FP16, tag="mlp_sg2")
                    nc.vector.tensor_tensor(out=sg2[:], in0=ps_g[:], in1=sg[:],
                                            op=ALU.mult)
                    ht = hpoolp.tile([128, 128], FP16, tag="mlp_h")
                    nc.vector.tensor_tensor(out=ht[:], in0=ps_y[:], in1=sg2[:],
                                            op=ALU.mult)
                    hh.append(ht)
                mT = mlpp.tile([128, 512], FP16, tag="mlp_mT")
                for ot in range(4):
                    ps = psmm.tile([128, 128], FP32, tag="ps")
                    for kt in range(n_h):
                        w = wmlpp.tile([128, 128], FP16, tag="wm2")
                        nc.sync.dma_start(
                            out=w[:], in_=w2_ap[kt][:, ot * 128:(ot + 1) * 128])
                        nc.tensor.matmul(ps[:], w[:], hh[kt][:],
                                         start=(kt == 0), stop=(kt == n_h - 1))
                    mo = mlpp.tile([128, 128], FP16, tag="mlp_mo")
                    nc.scalar.copy(out=mo[:], in_=ps[:])
                    pst = psmm.tile([128, 128], FP16, tag="ps")
                    nc.tensor.transpose(pst[:], mo[:], identT[:])
                    nc.scalar.copy(out=mT[:, ot * 128:(ot + 1) * 128], in_=pst[:])
                fin = mlpp.tile([128, 512], FP32, tag="mlp_fin")
                nc.vector.tensor_tensor(out=fin[:], in0=mT[:], in1=resid_tile[:],
                                        op=ALU.add)
                nc.sync.dma_start(out=out_d[out_row:out_row + 128, :], in_=fin[:])

            # ---------------- per-branch state
            ar_out = {}
            u_tiles = {}
            du_tiles = {}
            delta_all = {}
            carry = {}
            rs_in = {}
            rs_out = {}
            for m in range(3):
                L = BRANCHES[m][1]
                rs_in[m] = dramp.tile([L, 512], FP16, tag=f"rsin{m}",
                                      name=f"rsin{m}")
                for ch in range(L // 512):
                    rs_out[(m, ch)] = dramp.tile(
                        [128, 512], FP16, tag=f"rsout{m}_{ch}",
                        name=f"rsout{m}_{ch}")

            # ================= pre-scan stage
            def pre_branch(m):
                tok_off, L, n_ch = BRANCHES[m]
                wi = []
                for kt in range(4):
                    wt = wpool.tile([128, 512], FP16, tag="w_in")
                    nc.sync.dma_start(out=wt[:], in_=w_in[m][kt][:, 0:512])
                    wi.append(wt)
                wxp = []
                for kt in range(NDT):
                    wt = wpool.tile([128, 288], FP16, tag="w_xp")
                    nc.sync.dma_start(out=wt[:], in_=w_xp[m][kt])
                    wxp.append(wt)
                wdt = wpool.tile([DT_RANK, DI_SH], FP16, tag="w_dt")
                nc.sync.dma_start(out=wdt[:], in_=w_dt[m][:])
                wcv = []
                for dt in range(NDT):
                    wt = wpool.tile([128, D_CONV], FP32, tag="w_conv")
                    nc.sync.dma_start(out=wt[:], in_=w_conv[m][dt])
                    wcv.append(wt)

                xiT = [branchp.tile([128, L + 3], FP16, tag=f"xiT{dt}_{m}",
                                    name=f"xiT{dt}_{m}") for dt in range(NDT)]
                uu = [xiT[dt][:, 3:3 + L] for dt in range(NDT)]
                duu = [branchp.tile([128, L], FP16, tag=f"du{dt}_{m}",
                                    name=f"du{dt}_{m}") for dt in range(NDT)]
                dall = branchp.tile([128, NDT * L], FP16, tag=f"delta_{m}",
                                    name=f"delta_{m}")

                for dt in range(NDT):
                    nc.vector.memset(xiT[dt][:, 0:3], 0.0)

                # in_proj (xi half only; z recomputed at post time)
                for ch in range(n_ch):
                    c0 = tok_off + ch * F
                    for ot in range(4):
                        ps = psmm.tile([128, 512], FP32, tag="ps")
                        for kt in range(4):
                            nc.tensor.matmul(ps[:], wi[kt][:, ot * 128:(ot + 1) * 128],
                                             xnT[kt][:, c0:c0 + F],
                                             start=(kt == 0), stop=(kt == 3))
                        nc.scalar.copy(
                            out=xiT[ot][:, 3 + ch * F:3 + (ch + 1) * F], in_=ps[:])

                # conv + silu -> u
                for dt in range(NDT):
                    acc0 = small.tile([128, 1024], FP16, tag="conv_a", bufs=1)
                    acc1 = small.tile([128, 1024], FP16, tag="conv_b", bufs=1)
                    nc.vector.tensor_scalar(out=acc0[:, 0:L], in0=xiT[dt][:, 0:L],
                                            scalar1=wcv[dt][:, 0:1], scalar2=None,
                                            op0=ALU.mult)
                    a, b_ = acc0, acc1
                    for j in range(1, D_CONV):
                        nc.vector.scalar_tensor_tensor(
                            out=b_[:, 0:L], in0=xiT[dt][:, j:j + L],
                            scalar=wcv[dt][:, j:j + 1], in1=a[:, 0:L],
                            op0=ALU.mult, op1=ALU.add)
                        a, b_ = b_, a
                    nc.scalar.activation(out=b_[:, 0:L], in_=a[:, 0:L],
                                         func=AF.Sigmoid)
                    nc.vector.tensor_tensor(out=uu[dt][:, 0:L], in0=a[:, 0:L],
                                            in1=b_[:, 0:L], op=ALU.mult)

                # x_proj partials -> DRAM -> AllReduce (fp16)
                arin = dramp.tile([288, L], FP16, tag=f"arin{m}")
                arout = dramp.tile([288, L], FP16, tag=f"arout{m}")
                for ch in range(n_ch):
                    for po, pw in ((0, 128), (128, 128), (256, 32)):
                        ps = psmm.tile([128, 512], FP32, tag="ps")
                        for kt in range(NDT):
                            nc.tensor.matmul(
                                ps[:pw, :], wxp[kt][:, po:po + pw],
                                uu[kt][:, ch * F:(ch + 1) * F],
                                start=(kt == 0), stop=(kt == NDT - 1))
                        sb = small.tile([128, 512], FP16, tag="xdb_sb")
                        nc.scalar.copy(out=sb[:pw, :], in_=ps[:pw, :])
                        nc.sync.dma_start(
                            out=arin[po:po + pw, ch * F:(ch + 1) * F],
                            in_=sb[:pw, :])
                nc.gpsimd.collective_compute(
                    "AllReduce", ALU.add, replica_groups=GROUPS,
                    ins=[arin.opt()], outs=[arout.opt()])

                # dt_proj + softplus -> delta ; du = delta * u
                dtT = small.tile([DT_RANK, 1024], FP16, tag="dtT", bufs=1)
                nc.sync.dma_start(out=dtT[:, 0:L], in_=arout[0:DT_RANK, :])
                for dt in range(NDT):
                    for ch in range(n_ch):
                        ps = psmm.tile([128, 512], FP32, tag="ps")
                        nc.tensor.matmul(ps[:], wdt[:, dt * 128:(dt + 1) * 128],
                                         dtT[:, ch * F:(ch + 1) * F],
                                         start=True, stop=True)
                        spe = small.tile([128, 512], FP16, tag="spe", bufs=1)
                        nc.scalar.activation(out=spe[:], in_=ps[:], func=AF.Exp,
                                             bias=dtb_t[(m, dt)][:])
                        nc.scalar.activation(
                            out=dall[:, dt * L + ch * F: dt * L + (ch + 1) * F],
                            in_=spe[:], func=AF.Ln, bias=1.0)
                    nc.vector.tensor_tensor(out=duu[dt][:, 0:L],
                                            in0=dall[:, dt * L:dt * L + L],
                                            in1=uu[dt][:, 0:L], op=ALU.mult)

                ar_out[m] = arout
                u_tiles[m] = uu
                du_tiles[m] = duu
                delta_all[m] = dall
                if m == 2:
                    carry[m] = [branchp.tile([128, 128], FP16, tag=f"carry{dt}",
                                             name=f"carry{dt}")
                                for dt in range(NDT)]

            # ================= scan + post of one chunk
            def scan_chunk(m, ch):
                tok_off, L, n_ch = BRANCHES[m]
                arout = ar_out[m]
                dall = delta_all[m]
                duu = du_tiles[m]
                chained = (m == 2 and ch == 1)
                save_carry = (m == 2 and ch == 0)

                y_ps = [ypsum.tile([128, 512], FP32, tag=f"yps{dt}",
                                   name=f"yps{dt}_{m}_{ch}")
                        for dt in range(NDT)]
                for ng in range(NG):
                    n0 = ng * GN
                    b_bc = bcp.tile([128, BLK], FP16, tag="b_bc")
                    src = arout[DT_RANK + n0:DT_RANK + n0 + GN,
                                ch * F:(ch + 1) * F]
                    nc.sync.dma_start(
                        out=b_bc[:].rearrange("p (g f) -> p g f", g=GN),
                        in_=src.unsqueeze(0).broadcast_to([128, GN, F]))
                    c_bc = bcp.tile([128, BLK], FP16, tag="c_bc")
                    src = arout[DT_RANK + D_STATE + n0:
                                DT_RANK + D_STATE + n0 + GN,
                                ch * F:(ch + 1) * F]
                    nc.sync.dma_start(
                        out=c_bc[:].rearrange("p (g f) -> p g f", g=GN),
                        in_=src.unsqueeze(0).broadcast_to([128, GN, F]))

                    for dt in range(NDT):
                        dA = scanp.tile([128, BLK], FP16, tag="dA", bufs=2)
                        dA_g = dA[:].rearrange("p (g f) -> p g f", g=GN)
                        for j in range(GN):
                            nc.scalar.activation(
                                out=dA_g[:, j, :],
                                in_=dall[:, dt * L + ch * F:
                                         dt * L + (ch + 1) * F],
                                func=AF.Exp,
                                scale=A_t[dt][:, n0 + j:n0 + j + 1])
                        bb = scanp.tile([128, BLK], FP16, tag="bb")
                        du_view = duu[dt][:, ch * F:(ch + 1) * F]
                        nc.vector.tensor_tensor(
                            out=bb[:].rearrange("p (g f) -> p g f", g=GN),
                            in0=du_view.unsqueeze(1).broadcast_to([128, GN, F]),
                            in1=b_bc[:].rearrange("p (g f) -> p g f", g=GN),
                            op=ALU.mult)
                        if chained:
                            tmp = colsp.tile([128, GN], FP16, tag="fix")
                            nc.vector.tensor_tensor(
                                out=tmp[:], in0=dA_g[:, :, 0],
                                in1=carry[2][dt][:, n0:n0 + GN], op=ALU.mult)
                            bst = bb[:].rearrange("p (g f) -> p g f", g=GN)[:, :, 0]
                            nc.vector.tensor_tensor(out=bst, in0=tmp[:],
                                                    in1=bst, op=ALU.add)
                        nc.scalar.mul(out=dA_g[:, :, 0], in_=dA_g[:, :, 0],
                                      mul=0.0)
                        hh = scanp.tile([128, BLK], FP16, tag="hh")
                        nc.vector.tensor_tensor_scan(
                            out=hh[:], data0=dA[:], data1=bb[:], initial=0.0,
                            op0=ALU.mult, op1=ALU.add)
                        if save_carry:
                            nc.vector.tensor_copy(
                                out=carry[2][dt][:, n0:n0 + GN],
                                in_=hh[:].rearrange("p (g f) -> p g f",
                                                    g=GN)[:, :, F - 1])
                        prod = scanp.tile([128, BLK], FP16, tag="prod")
                        nc.vector.tensor_tensor(
                            out=prod[:], in0=hh[:], in1=c_bc[:], op=ALU.mult)
                        for g in range(GN):
                            nc.tensor.matmul(
                                y_ps[dt][:], identT[:],
                                prod[:, g * F:(g + 1) * F],
                                start=(ng == 0 and g == 0),
                                stop=(ng == NG - 1 and g == GN - 1))

                # ---- post: z -> silu ; y3 = (y + D*u) * silu(z) ; out_proj
                c0 = tok_off + ch * F
                wiz = []
                for kt in range(4):
                    wt = wpool.tile([128, 512], FP16, tag="w_z")
                    nc.sync.dma_start(out=wt[:], in_=w_in[m][kt][:, 512:1024])
                    wiz.append(wt)
                y3s = []
                for dt in range(NDT):
                    psz = psmm.tile([128, 512], FP32, tag="ps")
                    for kt in range(4):
                        nc.tensor.matmul(psz[:], wiz[kt][:, dt * 128:(dt + 1) * 128],
                                         xnT[kt][:, c0:c0 + F],
                                         start=(kt == 0), stop=(kt == 3))
                    sgz = postp.tile([128, 512], FP16, tag="sgz", bufs=1)
                    nc.scalar.activation(out=sgz[:], in_=psz[:], func=AF.Sigmoid)
                    szt = postp.tile([128, 512], FP16, tag="sz")
                    nc.vector.tensor_tensor(out=szt[:], in0=psz[:], in1=sgz[:],
                                            op=ALU.mult)
                    y2 = postp.tile([128, 512], FP16, tag="y2")
                    nc.vector.scalar_tensor_tensor(
                        out=y2[:], in0=u_tiles[m][dt][:, ch * F:(ch + 1) * F],
                        scalar=D_t[(m, dt)][:], in1=y_ps[dt][:],
                        op0=ALU.mult, op1=ALU.add)
                    y3 = postp.tile([128, 512], FP16, tag="y3", bufs=4)
                    nc.vector.tensor_tensor(out=y3[:], in0=y2[:], in1=szt[:],
                                            op=ALU.mult)
                    y3s.append(y3)

                wo = []
                for kt in range(NDT):
                    wt = wpool.tile([128, 512], FP16, tag="w_out")
                    nc.sync.dma_start(out=wt[:], in_=w_out[m][kt])
                    wo.append(wt)
                so = []
                for ot in range(4):
                    ps = psmm.tile([128, 512], FP32, tag="ps")
                    for kt in range(NDT):
                        nc.tensor.matmul(ps[:], wo[kt][:, ot * 128:(ot + 1) * 128],
                                         y3s[kt][:], start=(kt == 0),
                                         stop=(kt == NDT - 1))
                    st = postp.tile([128, 512], FP16, tag="so", bufs=4)
                    nc.scalar.copy(out=st[:], in_=ps[:])
                    so.append(st)
                for tt in range(4):
                    ob = postp.tile([128, 512], FP16, tag="obuf")
                    for ot in range(4):
                        pst = psmm.tile([128, 128], FP16, tag="ps")
                        nc.tensor.transpose(pst[:],
                                            so[ot][:, tt * 128:(tt + 1) * 128],
                                            identT[:])
                        nc.scalar.copy(out=ob[:, ot * 128:(ot + 1) * 128],
                                       in_=pst[:])
                    nc.sync.dma_start(
                        out=rs_in[m][ch * F + tt * 128:ch * F + (tt + 1) * 128, :],
                        in_=ob[:])

            # ================= post-RS MLP (one 512-token chunk of branch m)
            def mlp_branch(m, ch=0):
                nc.gpsimd.collective_compute(
                    "ReduceScatter", ALU.add, replica_groups=GROUPS,
                    ins=[rs_in[m][ch * 512:(ch + 1) * 512, :]],
                    outs=[rs_out[(m, ch)].opt()])
                sec0, nrow = OUT_SECS[m]
                rsv = mlpp.tile([128, 512], FP16, tag="rsv")
                nc.sync.dma_start(out=rsv[:], in_=rs_out[(m, ch)][:])
                xqv = mlpp.tile([128, 512], FP32, tag="xqv")
                nc.sync.dma_start(
                    out=xqv[:],
                    in_=xq[sec0 + ch * 128:sec0 + (ch + 1) * 128, :])
                resid = mlpp.tile([128, 512], FP32, tag="resid")
                nc.vector.tensor_tensor(out=resid[:], in0=rsv[:], in1=xqv[:],
                                        op=ALU.add)
                mlp_block(resid, w_fc1[m], HID_SMALL, w_fc2[m],
                          sec0 + ch * 128)

            # ---------------- schedule (program order guides the Tile
            # scheduler: each branch's AllReduce flies while the previous
            # branch's scan keeps the Vector engine saturated)
            pre_branch(0)
            xq_text = mlpp.tile([128, 512], FP32, tag="xq_text")
            nc.sync.dma_start(out=xq_text[:], in_=xq[512:640, :])
            mlp_block(xq_text, w_fc1_t, HID_TEXT, w_fc2_t, 512)
            pre_branch(1)
            scan_chunk(0, 0)
            pre_branch(2)
            scan_chunk(1, 0)
            scan_chunk(2, 0)
            mlp_branch(0)
            mlp_branch(1)
            mlp_branch(2, 0)
            scan_chunk(2, 1)
            mlp_branch(2, 1)

    nc.finalize()
    return nc


_PROGRAM = None


def _get_program():
    global _PROGRAM
    if _PROGRAM is None:
        _PROGRAM = build_program()
    return _PROGRAM


def _np(a, dt=np.float32):
    return np.ascontiguousarray(np.asarray(a), dtype=dt)


def prepare_in_maps(x, params):
    x = _np(x)
    norms = {k: _np(v) for k, v in params["norms"].items()}
    mamba = [{k: _np(v) for k, v in params[mk].items()} for mk in MAMBA_KEYS]
    mlps = [{k: _np(v) for k, v in params[mk].items()} for mk in MLP_KEYS]
    textp = {k: _np(v) for k, v in params["text_mlp"].items()}
    ident = np.eye(128, dtype=np.float16)
    norm_fold = [
        (norms["music_1"], norms["music_2"]),
        (norms["up_1"], norms["up_2"]),
        (norms["down_1"], norms["down_2"]),
    ]
    A_neg = -np.exp(mamba[0]["A_log"])
    for mmx in mamba[1:]:
        assert np.allclose(-np.exp(mmx["A_log"]), A_neg), \
            "mamba A_log differ across branches; not supported"
    for mm in mamba:
        assert np.all(mm["conv_b"] == 0), "conv bias not folded"
    for gm in mlps + [textp]:
        assert np.all(gm["fc1_b"] == 0) and np.all(gm["fc2_b"] == 0)

    in_maps = []
    for g in range(2):
        xg = x[g]
        x_t16 = np.ascontiguousarray(
            xg.astype(np.float16).reshape(16, 128, 512))
        for r in range(R_SH):
            d0 = r * DI_SH
            im = {"x_t": x_t16, "ident": ident}
            im["xq"] = np.ascontiguousarray(np.concatenate([
                xg[r * 128:(r + 1) * 128],
                xg[512 + r * 128:512 + (r + 1) * 128],
                xg[1024 + r * 128:1024 + (r + 1) * 128],
                xg[1536 + r * 128:1536 + (r + 1) * 128],
                xg[1536 + r * 128:1536 + (r + 1) * 128],
            ], axis=0), dtype=np.float32)
            im["A_bc"] = np.ascontiguousarray(
                A_neg[d0:d0 + DI_SH].reshape(NDT, 128, 128), dtype=np.float32)
            im["dtb"] = np.ascontiguousarray(
                np.stack([mamba[m]["dt_proj_b"][d0:d0 + DI_SH]
                          .reshape(NDT, 128, 1) for m in range(3)]),
                dtype=np.float32)
            im["D_sh"] = np.ascontiguousarray(
                np.stack([mamba[m]["D"][d0:d0 + DI_SH]
                          .reshape(NDT, 128, 1) for m in range(3)]),
                dtype=np.float32)
            for m in range(3):
                mm = mamba[m]
                w1n, w2n = norm_fold[m]
                Win = mm["in_proj_w"] * w1n[None, :]
                Wsl = np.concatenate(
                    [Win[d0:d0 + DI_SH],
                     Win[D_INNER + d0:D_INNER + d0 + DI_SH]], axis=0)
                im[f"w_in_{m}"] = np.ascontiguousarray(
                    Wsl.T.reshape(4, 128, 1024), dtype=np.float16)
                im[f"w_conv_{m}"] = np.ascontiguousarray(
                    mm["conv_w"][d0:d0 + DI_SH].reshape(NDT, 128, D_CONV),
                    dtype=np.float32)
                im[f"w_xp_{m}"] = np.ascontiguousarray(
                    mm["x_proj_w"][:, d0:d0 + DI_SH].T.reshape(NDT, 128, 288),
                    dtype=np.float16)
                im[f"w_dt_{m}"] = np.ascontiguousarray(
                    mm["dt_proj_w"][d0:d0 + DI_SH].T, dtype=np.float16)
                im[f"w_out_{m}"] = np.ascontiguousarray(
                    mm["out_proj_w"][:, d0:d0 + DI_SH].T.reshape(NDT, 128, 512),
                    dtype=np.float16)
                gm = mlps[m]
                F1 = gm["fc1_w"] * w2n[None, :]
                im[f"w_fc1_{m}"] = np.ascontiguousarray(
                    F1.T.reshape(4, 128, 2 * HID_SMALL), dtype=np.float16)
                im[f"w_fc2_{m}"] = np.ascontiguousarray(
                    gm["fc2_w"].T.reshape(HID_SMALL // 128, 128, 512),
                    dtype=np.float16)
            tw = norms[TEXT_SEGS[r]]
            T1 = textp["fc1_w"] * tw[None, :]
            im["w_fc1_t"] = np.ascontiguousarray(
                T1.T.reshape(4, 128, 2 * HID_TEXT), dtype=np.float16)
            im["w_fc2_t"] = np.ascontiguousarray(
                textp["fc2_w"].T.reshape(HID_TEXT // 128, 128, 512),
                dtype=np.float16)
            in_maps.append(im)
    return in_maps


def assemble(results):
    out = np.zeros((2, OUT_TOTAL, D_MODEL), dtype=np.float32)
    for g in range(2):
        for r in range(R_SH):
            o = results[g * R_SH + r]["out"]
            out[g, r * 128:(r + 1) * 128] = o[0:128]
            out[g, 512 + r * 128:512 + (r + 1) * 128] = o[128:256]
            out[g, 1024 + r * 128:1024 + (r + 1) * 128] = o[256:384]
            out[g, 1536 + r * 128:1536 + (r + 1) * 128] = o[384:512]
            out[g, 2048 + r * 128:2048 + (r + 1) * 128] = o[512:640]
    return out


def kernel(x, T_motion, text_meta, params):
    assert int(T_motion) == T, f"kernel compiled for T_motion={T}"
    nc = _get_program()
    in_maps = prepare_in_maps(x, params)
    res = run_bass_kernel_spmd(nc, in_maps, core_ids=list(range(8)))
    return assemble(res.results)


# revision 29
# speedup vs baseline: 1.0185x; 1.0042x over previous
"""Trainium2 Bass kernel for nn_Block_Head_83047487635490 (Mamba motion block).

Sharding: 8 cores = 2 batch groups x 4-way tensor-parallel (d_inner 2048 -> 512
per core).  Per group: mamba in_proj/conv/x_proj/dt_proj/scan/out_proj sharded
over d_inner with a small fp16 all-reduce of x_db (288 x L) and a
reduce-scatter of out_proj partials over tokens; the gated MLPs and the text
MLP run token-parallel (full hidden) on the scattered quarters.

Selective scan: per (chunk, dtile, n-group) the recurrence
h = exp(delta*A_n) * h + (delta*u) * B_n runs as a single fp16
tensor_tensor_scan over Gn concatenated n-blocks (decay zeroed at block
starts resets the state); y = sum_n h*C_n via PE identity-matmul accumulation
in PSUM.
"""

import sys

if "/opt/trn_rl_repo" not in sys.path:
    sys.path.insert(0, "/opt/trn_rl_repo")

import numpy as np

from concourse import bacc, bass, mybir, tile
from concourse.bass_utils import run_bass_kernel_spmd

# ---------------------------------------------------------------- constants
B = 2
T = 512
N_TEXT = 128
TOTAL = 2048          # 3*T + 4*N_TEXT
D_MODEL = 512
D_STATE = 128
D_CONV = 4
D_INNER = 2048
DT_RANK = 32
HID_SMALL = 768
HID_TEXT = 2048
EPS = 1e-6
OUT_TOTAL = 2560      # music 512 + up 512 + down 1024 + text 512

R_SH = 4              # TP degree
DI_SH = D_INNER // R_SH
NDT = DI_SH // 128    # 4 d-tiles per core
F = 512               # time chunk
GN = 4                # n values per scan instruction
NG = D_STATE // GN    # 32
BLK = GN * F

BRANCHES = [(0, 512, 1), (512, 512, 1), (1024, 1024, 2)]  # (tok_off, L, chunks)
MLP_KEYS = ["gate_mlp_1", "gate_mlp_2", "gate_mlp_3"]
TEXT_SEGS = ["text_upper", "text_lower", "text_torso", "text_whole"]
MAMBA_KEYS = ["mamba_music", "mamba_up", "mamba_down"]

FP16 = mybir.dt.float16
FP32 = mybir.dt.float32
AF = mybir.ActivationFunctionType
ALU = mybir.AluOpType

OUT_SECS = [(0, 128), (128, 128), (256, 256), (512, 128)]
GROUPS = [[0, 1, 2, 3], [4, 5, 6, 7]]


def build_program():
    nc = bacc.Bacc("TRN2", target_bir_lowering=False, debug=False, num_devices=8)

    def din(name, shape, dt=FP16):
        return nc.dram_tensor(name, list(shape), dt, kind="ExternalInput").ap()

    x_t = din("x_t", (16, 128, 512))
    xq = din("xq", (640, 512), FP32)
    ident_in = din("ident", (128, 128))
    A_in = din("A_bc", (NDT, 128, 128), FP32)
    dtb_in = din("dtb", (3, NDT, 128, 1), FP32)
    D_in = din("D_sh", (3, NDT, 128, 1), FP32)
    w_in = [din(f"w_in_{m}", (4, 128, 1024)) for m in range(3)]
    w_conv = [din(f"w_conv_{m}", (NDT, 128, D_CONV), FP32) for m in range(3)]
    w_xp = [din(f"w_xp_{m}", (NDT, 128, 288)) for m in range(3)]
    w_dt = [din(f"w_dt_{m}", (DT_RANK, DI_SH)) for m in range(3)]
    w_out = [din(f"w_out_{m}", (NDT, 128, 512)) for m in range(3)]
    w_fc1 = [din(f"w_fc1_{m}", (4, 128, 2 * HID_SMALL)) for m in range(3)]
    w_fc2 = [din(f"w_fc2_{m}", (HID_SMALL // 128, 128, 512)) for m in range(3)]
    w_fc1_t = din("w_fc1_t", (4, 128, 2 * HID_TEXT))
    w_fc2_t = din("w_fc2_t", (HID_TEXT // 128, 128, 512))
    out_d = nc.dram_tensor("out", [640, 512], FP32, kind="ExternalOutput").ap()

    with tile.TileContext(nc) as tc:
        with (
            tc.tile_pool(name="const", bufs=1) as constp,
            tc.tile_pool(name="persist", bufs=1) as persist,
            tc.tile_pool(name="small", bufs=2) as small,
            tc.tile_pool(name="cols", bufs=4) as colsp,
            tc.tile_pool(name="wpool", bufs=4) as wpool,
            tc.tile_pool(name="wmlp", bufs=4) as wmlpp,
            tc.tile_pool(name="branch", bufs=1) as branchp,
            tc.tile_pool(name="scan", bufs=3) as scanp,
            tc.tile_pool(name="bc", bufs=4) as bcp,
            tc.tile_pool(name="post", bufs=2) as postp,
            tc.tile_pool(name="mlp", bufs=1) as mlpp,
            tc.tile_pool(name="hpool", bufs=16) as hpoolp,
            tc.tile_pool(name="psmm", bufs=4, space="PSUM") as psmm,
            tc.tile_pool(name="ypsum", bufs=1, space="PSUM") as ypsum,
            tc.tile_pool(name="dram", bufs=1, space="DRAM") as dramp,
        ):
            # ---------------- constants
            identT = constp.tile([128, 128], FP16, tag="ident")
            nc.sync.dma_start(out=identT[:], in_=ident_in[:])
            epscol = constp.tile([128, 1], FP32, tag="epscol")
            nc.vector.memset(epscol[:], EPS)
            A_t, dtb_t, D_t = [], {}, {}
            for dt in range(NDT):
                a = constp.tile([128, 128], FP32, tag=f"A{dt}")
                nc.sync.dma_start(out=a[:], in_=A_in[dt])
                A_t.append(a)
            for m in range(3):
                for dt in range(NDT):
                    bcol = constp.tile([128, 1], FP32, tag=f"dtb{m}_{dt}")
                    nc.sync.dma_start(out=bcol[:], in_=dtb_in[m, dt])
                    dtb_t[(m, dt)] = bcol
                    dcol = constp.tile([128, 1], FP32, tag=f"D{m}_{dt}")
                    nc.sync.dma_start(out=dcol[:], in_=D_in[m, dt])
                    D_t[(m, dt)] = dcol

            # ---------------- helpers
            def rms_tile(src_tile, dst_tile):
                scratch = small.tile([128, 512], FP16, tag="rms_scratch", bufs=1)
                ssq = colsp.tile([128, 1], FP32, tag="rms_ssq")
                nc.scalar.activation(out=scratch[:], in_=src_tile[:],
                                     func=AF.Square, accum_out=ssq[:])
                rr = colsp.tile([128, 1], FP32, tag="rms_rr")
                nc.scalar.activation(out=rr[:], in_=ssq[:], func=AF.Sqrt,
                                     scale=1.0 / D_MODEL, bias=epscol[:])
                inv = colsp.tile([128, 1], FP32, tag="rms_inv")
                nc.vector.reciprocal(out=inv[:], in_=rr[:])
                nc.vector.tensor_scalar(out=dst_tile[:], in0=src_tile[:],
                                        scalar1=inv[:], scalar2=None,
                                        op0=ALU.mult)

            def transpose_into(src_tile, n_blocks, put_block):
                for bb in range(n_blocks):
                    pst = psmm.tile([128, 128], FP16, tag="ps")
                    nc.tensor.transpose(pst[:], src_tile[:, bb * 128:(bb + 1) * 128],
                                        identT[:])
                    put_block(bb, pst)

            # ---------------- prolog: x -> rmsnorm -> transpose -> xnT
            xnT = [persist.tile([128, TOTAL], FP16, tag=f"xnT{k}",
                                name=f"xnT{k}") for k in range(4)]
            for i in range(16):
                xt = postp.tile([128, 512], FP16, tag="so", bufs=4, name="xt")
                nc.sync.dma_start(out=xt[:], in_=x_t[i])
                xn = postp.tile([128, 512], FP16, tag="y2", name="xn")
                rms_tile(xt, xn)

                def put(kt, pst, i=i):
                    nc.scalar.copy(out=xnT[kt][:, i * 128:(i + 1) * 128],
                                   in_=pst[:])
                transpose_into(xn, 4, put)

            # ---------------- generic gated-MLP block on one (128, 512) resid
            def mlp_block(resid_tile, w1_ap, hid, w2_ap, out_row):
                xn2 = mlpp.tile([128, 512], FP16, tag="mlp_xn")
                rms_tile(resid_tile, xn2)
                xn2T = []
                for kt in range(4):
                    dstt = mlpp.tile([128, 128], FP16, tag=f"mlp_xnT{kt}")
                    xn2T.append(dstt)

                def putx(kt, pst):
                    nc.scalar.copy(out=xn2T[kt][:], in_=pst[:])
                transpose_into(xn2, 4, putx)

                n_h = hid // 128
                hh = []
                for oy in range(n_h):
                    ps_y = psmm.tile([128, 128], FP32, tag="ps")
                    ps_g = psmm.tile([128, 128], FP32, tag="ps")
                    for half, ps in ((0, ps_y), (1, ps_g)):
                        o = oy + half * n_h
                        for kt in range(4):
                            w = wmlpp.tile([128, 128], FP16, tag="wm1")
                            nc.sync.dma_start(
                                out=w[:], in_=w1_ap[kt][:, o * 128:(o + 1) * 128])
                            nc.tensor.matmul(ps[:], w[:], xn2T[kt][:],
                                             start=(kt == 0), stop=(kt == 3))
                    sg = mlpp.tile([128, 128], FP16, tag="mlp_sg")
                    nc.scalar.activation(out=sg[:], in_=ps_g[:], func=AF.Sigmoid)
                    sg2 = mlpp.tile([128, 128], FP16, tag="mlp_sg2")
                    nc.vector.tensor_tensor(out=sg2[:], in0=ps_g[:], in1=sg[:],
                                            op=ALU.mult)
                    ht = hpoolp.tile([128, 128], FP16, tag="mlp_h")
                    nc.vector.tensor_tensor(out=ht[:], in0=ps_y[:], in1=sg2[:],
                                            op=ALU.mult)
                    hh.append(ht)
                mT = mlpp.tile([128, 512], FP16, tag="mlp_mT")
                for ot in range(4):
                    ps = psmm.tile([128, 128], FP32, tag="ps")
                    for kt in range(n_h):
                        w = wmlpp.tile([128, 128], FP16, tag="wm2")
                        nc.sync.dma_start(
                            out=w[:], in_=w2_ap[kt][:, ot * 128:(ot + 1) * 128])
                        nc.tensor.matmul(ps[:], w[:], hh[kt][:],
                                         start=(kt == 0), stop=(kt == n_h - 1))
                    mo = mlpp.tile([128, 128], FP16, tag="mlp_mo")
                    nc.scalar.copy(out=mo[:], in_=ps[:])
                    pst = psmm.tile([128, 128], FP16, tag="ps")
                    nc.tensor.transpose(pst[:], mo[:], identT[:])
                    nc.scalar.copy(out=mT[:, ot * 128:(ot + 1) * 128], in_=pst[:])
                fin = mlpp.tile([128, 512], FP32, tag="mlp_fin")
                nc.vector.tensor_tensor(out=fin[:], in0=mT[:], in1=resid_tile[:],
                                        op=ALU.add)
                nc.sync.dma_start(out=out_d[out_row:out_row + 128, :], in_=fin[:])

            # ---------------- per-branch state
            ar_out = {}
            u_tiles = {}
            du_tiles = {}
            delta_all = {}
            carry = {}
            rs_in = {}
            rs_out = {}
            for m in range(3):
                L = BRANCHES[m][1]
                rs_in[m] = dramp.tile([L, 512], FP16, tag=f"rsin{m}",
                                      name=f"rsin{m}")
                for ch in range(L // 512):
                    rs_out[(m, ch)] = dramp.tile(
                        [128, 512], FP16, tag=f"rsout{m}_{ch}",
                        name=f"rsout{m}_{ch}")

            # ================= pre-scan stage
            def pre_branch(m):
                tok_off, L, n_ch = BRANCHES[m]
                wi = []
                for kt in range(4):
                    wt = wpool.tile([128, 512], FP16, tag="w_in")
                    nc.sync.dma_start(out=wt[:], in_=w_in[m][kt][:, 0:512])
                    wi.append(wt)
                wxp = []
                for kt in range(NDT):
                    wt = wpool.tile([128, 288], FP16, tag="w_xp")
                    nc.sync.dma_start(out=wt[:], in_=w_xp[m][kt])
                    wxp.append(wt)
                wdt = wpool.tile([DT_RANK, DI_SH], FP16, tag="w_dt")
                nc.sync.dma_start(out=wdt[:], in_=w_dt[m][:])
                wcv = []
                for dt in range(NDT):
                    wt = wpool.tile([128, D_CONV], FP32, tag="w_conv")
                    nc.sync.dma_start(out=wt[:], in_=w_conv[m][dt])
                    wcv.append(wt)

                xiT = [branchp.tile([128, L + 3], FP16, tag=f"xiT{dt}_{m}",
                                    name=f"xiT{dt}_{m}") for dt in range(NDT)]
                uu = [xiT[dt][:, 3:3 + L] for dt in range(NDT)]
                duu = [branchp.tile([128, L], FP16, tag=f"du{dt}_{m % 2}",
                                    name=f"du{dt}_{m}") for dt in range(NDT)]
                dall = branchp.tile([128, NDT * L], FP16, tag=f"delta_{m % 2}",
                                    name=f"delta_{m}")

                for dt in range(NDT):
                    nc.vector.memset(xiT[dt][:, 0:3], 0.0)

                # in_proj (xi half only; z recomputed at post time)
                for ch in range(n_ch):
                    c0 = tok_off + ch * F
                    for ot in range(4):
                        ps = psmm.tile([128, 512], FP32, tag="ps")
                        for kt in range(4):
                            nc.tensor.matmul(ps[:], wi[kt][:, ot * 128:(ot + 1) * 128],
                                             xnT[kt][:, c0:c0 + F],
                                             start=(kt == 0), stop=(kt == 3))
                        nc.scalar.copy(
                            out=xiT[ot][:, 3 + ch * F:3 + (ch + 1) * F], in_=ps[:])

                # conv + silu -> u
                for dt in range(NDT):
                    acc0 = small.tile([128, 1024], FP16, tag="conv_a", bufs=1)
                    acc1 = small.tile([128, 1024], FP16, tag="conv_b", bufs=1)
                    nc.vector.tensor_scalar(out=acc0[:, 0:L], in0=xiT[dt][:, 0:L],
                                            scalar1=wcv[dt][:, 0:1], scalar2=None,
                                            op0=ALU.mult)
                    a, b_ = acc0, acc1
                    for j in range(1, D_CONV):
                        nc.vector.scalar_tensor_tensor(
                            out=b_[:, 0:L], in0=xiT[dt][:, j:j + L],
                            scalar=wcv[dt][:, j:j + 1], in1=a[:, 0:L],
                            op0=ALU.mult, op1=ALU.add)
                        a, b_ = b_, a
                    nc.scalar.activation(out=b_[:, 0:L], in_=a[:, 0:L],
                                         func=AF.Sigmoid)
                    nc.vector.tensor_tensor(out=uu[dt][:, 0:L], in0=a[:, 0:L],
                                            in1=b_[:, 0:L], op=ALU.mult)

                # x_proj partials -> DRAM -> AllReduce (fp16)
                arin = dramp.tile([288, L], FP16, tag=f"arin{m}")
                arout = dramp.tile([288, L], FP16, tag=f"arout{m}")
                for ch in range(n_ch):
                    for po, pw in ((0, 128), (128, 128), (256, 32)):
                        ps = psmm.tile([128, 512], FP32, tag="ps")
                        for kt in range(NDT):
                            nc.tensor.matmul(
                                ps[:pw, :], wxp[kt][:, po:po + pw],
                                uu[kt][:, ch * F:(ch + 1) * F],
                                start=(kt == 0), stop=(kt == NDT - 1))
                        sb = small.tile([128, 512], FP16, tag="xdb_sb")
                        nc.scalar.copy(out=sb[:pw, :], in_=ps[:pw, :])
                        nc.sync.dma_start(
                            out=arin[po:po + pw, ch * F:(ch + 1) * F],
                            in_=sb[:pw, :])
                nc.gpsimd.collective_compute(
                    "AllReduce", ALU.add, replica_groups=GROUPS,
                    ins=[arin.opt()], outs=[arout.opt()])

                # dt_proj + softplus -> delta ; du = delta * u
                dtT = small.tile([DT_RANK, 1024], FP16, tag="dtT", bufs=1)
                nc.sync.dma_start(out=dtT[:, 0:L], in_=arout[0:DT_RANK, :])
                for dt in range(NDT):
                    for ch in range(n_ch):
                        ps = psmm.tile([128, 512], FP32, tag="ps")
                        nc.tensor.matmul(ps[:], wdt[:, dt * 128:(dt + 1) * 128],
                                         dtT[:, ch * F:(ch + 1) * F],
                                         start=True, stop=True)
                        spe = small.tile([128, 512], FP16, tag="spe", bufs=1)
                        nc.scalar.activation(out=spe[:], in_=ps[:], func=AF.Exp,
                                             bias=dtb_t[(m, dt)][:])
                        nc.scalar.activation(
                            out=dall[:, dt * L + ch * F: dt * L + (ch + 1) * F],
                            in_=spe[:], func=AF.Ln, bias=1.0)
                    nc.vector.tensor_tensor(out=duu[dt][:, 0:L],
                                            in0=dall[:, dt * L:dt * L + L],
                                            in1=uu[dt][:, 0:L], op=ALU.mult)

                ar_out[m] = arout
                u_tiles[m] = uu
                du_tiles[m] = duu
                delta_all[m] = dall
                if m == 2:
                    carry[m] = [branchp.tile([128, 128], FP16, tag=f"carry{dt}",
                                             name=f"carry{dt}")
                                for dt in range(NDT)]

            # ================= scan + post of one chunk
            def scan_chunk(m, ch):
                tok_off, L, n_ch = BRANCHES[m]
                arout = ar_out[m]
                dall = delta_all[m]
                duu = du_tiles[m]
                chained = (m == 2 and ch == 1)
                save_carry = (m == 2 and ch == 0)

                y_ps = [ypsum.tile([128, 512], FP32, tag=f"yps{dt}",
                                   name=f"yps{dt}_{m}_{ch}")
                        for dt in range(NDT)]
                for ng in range(NG):
                    n0 = ng * GN
                    b_bc = bcp.tile([128, BLK], FP16, tag="b_bc")
                    src = arout[DT_RANK + n0:DT_RANK + n0 + GN,
                                ch * F:(ch + 1) * F]
                    nc.sync.dma_start(
                        out=b_bc[:].rearrange("p (g f) -> p g f", g=GN),
                        in_=src.unsqueeze(0).broadcast_to([128, GN, F]))
                    c_bc = bcp.tile([128, BLK], FP16, tag="c_bc")
                    src = arout[DT_RANK + D_STATE + n0:
                                DT_RANK + D_STATE + n0 + GN,
                                ch * F:(ch + 1) * F]
                    nc.sync.dma_start(
                        out=c_bc[:].rearrange("p (g f) -> p g f", g=GN),
                        in_=src.unsqueeze(0).broadcast_to([128, GN, F]))

                    for dt in range(NDT):
                        dA = scanp.tile([128, BLK], FP16, tag="dA", bufs=2)
                        dA_g = dA[:].rearrange("p (g f) -> p g f", g=GN)
                        for j in range(GN):
                            nc.scalar.activation(
                                out=dA_g[:, j, :],
                                in_=dall[:, dt * L + ch * F:
                                         dt * L + (ch + 1) * F],
                                func=AF.Exp,
                                scale=A_t[dt][:, n0 + j:n0 + j + 1])
                        bb = scanp.tile([128, BLK], FP16, tag="bb", bufs=4)
                        du_view = duu[dt][:, ch * F:(ch + 1) * F]
                        nc.vector.tensor_tensor(
                            out=bb[:].rearrange("p (g f) -> p g f", g=GN),
                            in0=du_view.unsqueeze(1).broadcast_to([128, GN, F]),
                            in1=b_bc[:].rearrange("p (g f) -> p g f", g=GN),
                            op=ALU.mult)
                        if chained:
                            tmp = colsp.tile([128, GN], FP16, tag="fix")
                            nc.vector.tensor_tensor(
                                out=tmp[:], in0=dA_g[:, :, 0],
                                in1=carry[2][dt][:, n0:n0 + GN], op=ALU.mult)
                            bst = bb[:].rearrange("p (g f) -> p g f", g=GN)[:, :, 0]
                            nc.vector.tensor_tensor(out=bst, in0=tmp[:],
                                                    in1=bst, op=ALU.add)
                        nc.scalar.mul(out=dA_g[:, :, 0], in_=dA_g[:, :, 0],
                                      mul=0.0)
                        hh = scanp.tile([128, BLK], FP16, tag="hh")
                        nc.vector.tensor_tensor_scan(
                            out=hh[:], data0=dA[:], data1=bb[:], initial=0.0,
                            op0=ALU.mult, op1=ALU.add)
                        if save_carry:
                            nc.vector.tensor_copy(
                                out=carry[2][dt][:, n0:n0 + GN],
                                in_=hh[:].rearrange("p (g f) -> p g f",
                                                    g=GN)[:, :, F - 1])
                        prod = scanp.tile([128, BLK], FP16, tag="prod", bufs=4)
                        nc.vector.tensor_tensor(
                            out=prod[:], in0=hh[:], in1=c_bc[:], op=ALU.mult)
                        for g in range(GN):
                            nc.tensor.matmul(
                                y_ps[dt][:], identT[:],
                                prod[:, g * F:(g + 1) * F],
                                start=(ng == 0 and g == 0),
                                stop=(ng == NG - 1 and g == GN - 1))

                # ---- post: z -> silu ; y3 = (y + D*u) * silu(z) ; out_proj
                c0 = tok_off + ch * F
                wiz = []
                for kt in range(4):
                    wt = wpool.tile([128, 512], FP16, tag="w_z")
                    nc.sync.dma_start(out=wt[:], in_=w_in[m][kt][:, 512:1024])
                    wiz.append(wt)
                y3s = []
                for dt in range(NDT):
                    psz = psmm.tile([128, 512], FP32, tag="ps")
                    for kt in range(4):
                        nc.tensor.matmul(psz[:], wiz[kt][:, dt * 128:(dt + 1) * 128],
                                         xnT[kt][:, c0:c0 + F],
                                         start=(kt == 0), stop=(kt == 3))
                    sgz = postp.tile([128, 512], FP16, tag="sgz", bufs=1)
                    nc.scalar.activation(out=sgz[:], in_=psz[:], func=AF.Sigmoid)
                    szt = postp.tile([128, 512], FP16, tag="sz")
                    nc.vector.tensor_tensor(out=szt[:], in0=psz[:], in1=sgz[:],
                                            op=ALU.mult)
                    y2 = postp.tile([128, 512], FP16, tag="y2")
                    nc.vector.scalar_tensor_tensor(
                        out=y2[:], in0=u_tiles[m][dt][:, ch * F:(ch + 1) * F],
                        scalar=D_t[(m, dt)][:], in1=y_ps[dt][:],
                        op0=ALU.mult, op1=ALU.add)
                    y3 = postp.tile([128, 512], FP16, tag="y3", bufs=4)
                    nc.vector.tensor_tensor(out=y3[:], in0=y2[:], in1=szt[:],
                                            op=ALU.mult)
                    y3s.append(y3)

                wo = []
                for kt in range(NDT):
                    wt = wpool.tile([128, 512], FP16, tag="w_out")
                    nc.sync.dma_start(out=wt[:], in_=w_out[m][kt])
                    wo.append(wt)
                so = []
                for ot in range(4):
                    ps = psmm.tile([128, 512], FP32, tag="ps")
                    for kt in range(NDT):
                        nc.tensor.matmul(ps[:], wo[kt][:, ot * 128:(ot + 1) * 128],
                                         y3s[kt][:], start=(kt == 0),
                                         stop=(kt == NDT - 1))
                    st = postp.tile([128, 512], FP16, tag="so", bufs=4)
                    nc.scalar.copy(out=st[:], in_=ps[:])
                    so.append(st)
                for tt in range(4):
                    ob = postp.tile([128, 512], FP16, tag="obuf")
                    for ot in range(4):
                        pst = psmm.tile([128, 128], FP16, tag="ps")
                        nc.tensor.transpose(pst[:],
                                            so[ot][:, tt * 128:(tt + 1) * 128],
                                            identT[:])
                        nc.scalar.copy(out=ob[:, ot * 128:(ot + 1) * 128],
                                       in_=pst[:])
                    nc.sync.dma_start(
                        out=rs_in[m][ch * F + tt * 128:ch * F + (tt + 1) * 128, :],
                        in_=ob[:])

            # ================= post-RS MLP (one 512-token chunk of branch m)
            def mlp_branch(m, ch=0):
                nc.gpsimd.collective_compute(
                    "ReduceScatter", ALU.add, replica_groups=GROUPS,
                    ins=[rs_in[m][ch * 512:(ch + 1) * 512, :]],
                    outs=[rs_out[(m, ch)].opt()])
                sec0, nrow = OUT_SECS[m]
                rsv = mlpp.tile([128, 512], FP16, tag="rsv")
                nc.sync.dma_start(out=rsv[:], in_=rs_out[(m, ch)][:])
                xqv = mlpp.tile([128, 512], FP32, tag="xqv")
                nc.sync.dma_start(
                    out=xqv[:],
                    in_=xq[sec0 + ch * 128:sec0 + (ch + 1) * 128, :])
                resid = mlpp.tile([128, 512], FP32, tag="resid")
                nc.vector.tensor_tensor(out=resid[:], in0=rsv[:], in1=xqv[:],
                                        op=ALU.add)
                mlp_block(resid, w_fc1[m], HID_SMALL, w_fc2[m],
                          sec0 + ch * 128)

            # ---------------- schedule (program order guides the Tile
            # scheduler: each branch's AllReduce flies while the previous
            # branch's scan keeps the Vector engine saturated)
            pre_branch(0)
            xq_text = mlpp.tile([128, 512], FP32, tag="xq_text")
            nc.sync.dma_start(out=xq_text[:], in_=xq[512:640, :])
            mlp_block(xq_text, w_fc1_t, HID_TEXT, w_fc2_t, 512)
            pre_branch(1)
            scan_chunk(0, 0)
            pre_branch(2)
            scan_chunk(1, 0)
            scan_chunk(2, 0)
            mlp_branch(0)
            mlp_branch(1)
            mlp_branch(2, 0)
            scan_chunk(2, 1)
            mlp_branch(2, 1)

    nc.finalize()
    return nc


_PROGRAM = None


def _get_program():
    global _PROGRAM
    if _PROGRAM is None:
        _PROGRAM = build_program()
    return _PROGRAM


def _np(a, dt=np.float32):
    return np.ascontiguousarray(np.asarray(a), dtype=dt)


def prepare_in_maps(x, params):
    x = _np(x)
    norms = {k: _np(v) for k, v in params["norms"].items()}
    mamba = [{k: _np(v) for k, v in params[mk].items()} for mk in MAMBA_KEYS]
    mlps = [{k: _np(v) for k, v in params[mk].items()} for mk in MLP_KEYS]
    textp = {k: _np(v) for k, v in params["text_mlp"].items()}
    ident = np.eye(128, dtype=np.float16)
    norm_fold = [
        (norms["music_1"], norms["music_2"]),
        (norms["up_1"], norms["up_2"]),
        (norms["down_1"], norms["down_2"]),
    ]
    A_neg = -np.exp(mamba[0]["A_log"])
    for mmx in mamba[1:]:
        assert np.allclose(-np.exp(mmx["A_log"]), A_neg), \
            "mamba A_log differ across branches; not supported"
    for mm in mamba:
        assert np.all(mm["conv_b"] == 0), "conv bias not folded"
    for gm in mlps + [textp]:
        assert np.all(gm["fc1_b"] == 0) and np.all(gm["fc2_b"] == 0)

    in_maps = []
    for g in range(2):
        xg = x[g]
        x_t16 = np.ascontiguousarray(
            xg.astype(np.float16).reshape(16, 128, 512))
        for r in range(R_SH):
            d0 = r * DI_SH
            im = {"x_t": x_t16, "ident": ident}
            im["xq"] = np.ascontiguousarray(np.concatenate([
                xg[r * 128:(r + 1) * 128],
                xg[512 + r * 128:512 + (r + 1) * 128],
                xg[1024 + r * 128:1024 + (r + 1) * 128],
                xg[1536 + r * 128:1536 + (r + 1) * 128],
                xg[1536 + r * 128:1536 + (r + 1) * 128],
            ], axis=0), dtype=np.float32)
            im["A_bc"] = np.ascontiguousarray(
                A_neg[d0:d0 + DI_SH].reshape(NDT, 128, 128), dtype=np.float32)
            im["dtb"] = np.ascontiguousarray(
                np.stack([mamba[m]["dt_proj_b"][d0:d0 + DI_SH]
                          .reshape(NDT, 128, 1) for m in range(3)]),
                dtype=np.float32)
            im["D_sh"] = np.ascontiguousarray(
                np.stack([mamba[m]["D"][d0:d0 + DI_SH]
                          .reshape(NDT, 128, 1) for m in range(3)]),
                dtype=np.float32)
            for m in range(3):
                mm = mamba[m]
                w1n, w2n = norm_fold[m]
                Win = mm["in_proj_w"] * w1n[None, :]
                Wsl = np.concatenate(
                    [Win[d0:d0 + DI_SH],
                     Win[D_INNER + d0:D_INNER + d0 + DI_SH]], axis=0)
                im[f"w_in_{m}"] = np.ascontiguousarray(
                    Wsl.T.reshape(4, 128, 1024), dtype=np.float16)
                im[f"w_conv_{m}"] = np.ascontiguousarray(
                    mm["conv_w"][d0:d0 + DI_SH].reshape(NDT, 128, D_CONV),
                    dtype=np.float32)
                im[f"w_xp_{m}"] = np.ascontiguousarray(
                    mm["x_proj_w"][:, d0:d0 + DI_SH].T.reshape(NDT, 128, 288),
                    dtype=np.float16)
                im[f"w_dt_{m}"] = np.ascontiguousarray(
                    mm["dt_proj_w"][d0:d0 + DI_SH].T, dtype=np.float16)
                im[f"w_out_{m}"] = np.ascontiguousarray(
                    mm["out_proj_w"][:, d0:d0 + DI_SH].T.reshape(NDT, 128, 512),
                    dtype=np.float16)
                gm = mlps[m]
                F1 = gm["fc1_w"] * w2n[None, :]
                im[f"w_fc1_{m}"] = np.ascontiguousarray(
                    F1.T.reshape(4, 128, 2 * HID_SMALL), dtype=np.float16)
                im[f"w_fc2_{m}"] = np.ascontiguousarray(
                    gm["fc2_w"].T.reshape(HID_SMALL // 128, 128, 512),
                    dtype=np.float16)
            tw = norms[TEXT_SEGS[r]]
            T1 = textp["fc1_w"] * tw[None, :]
            im["w_fc1_t"] = np.ascontiguousarray(
                T1.T.reshape(4, 128, 2 * HID_TEXT), dtype=np.float16)
            im["w_fc2_t"] = np.ascontiguousarray(
                textp["fc2_w"].T.reshape(HID_TEXT // 128, 128, 512),
                dtype=np.float16)
            in_maps.append(im)
    return in_maps


def assemble(results):
    out = np.zeros((2, OUT_TOTAL, D_MODEL), dtype=np.float32)
    for g in range(2):
        for r in range(R_SH):
            o = results[g * R_SH + r]["out"]
            out[g, r * 128:(r + 1) * 128] = o[0:128]
            out[g, 512 + r * 128:512 + (r + 1) * 128] = o[128:256]
            out[g, 1024 + r * 128:1024 + (r + 1) * 128] = o[256:384]
            out[g, 1536 + r * 128:1536 + (r + 1) * 128] = o[384:512]
            out[g, 2048 + r * 128:2048 + (r + 1) * 128] = o[512:640]
    return out


def kernel(x, T_motion, text_meta, params):
    assert int(T_motion) == T, f"kernel compiled for T_motion={T}"
    nc = _get_program()
    in_maps = prepare_in_maps(x, params)
    res = run_bass_kernel_spmd(nc, in_maps, core_ids=list(range(8)))
    return assemble(res.results)
